# revision 16
# baseline (speedup 1.0000x reference)
"""MultiHeadAttention Trainium2 kernel (8 NeuronCores, SPMD).

Reference computation (B=4, T=1024, D=768, H=12, Dh=64):
    q = x @ Wq.T ; k = x @ Wk.T ; v = x @ Wv.T       (per-head reshape)
    attn = softmax((q @ k.T) / 8)
    out = (attn @ v) @ Wo.T + bo
Sharding: 8 cores = 4 batches x 2 head-halves (6 heads each); host sums
the two partials per batch and adds the bias.

fp8 DoubleRow everywhere on the PE (0.5 cycles/output-column):
  - q: (hi, lo) compensation pair from a 2-term DR projection.
  - k: 1-term (w_hi only) DR projection, requantized to a duplicated
    single-fp8 pair (k feeds the scores matmul as fp8 anyway).
  - v: 3-term compensated projection (accuracy matters directly).
  - scores: DR with moving (q_hi, q_lo) pair vs stationary dup-k pair,
    fp32 PSUM tiles [128, 1024] (TRN2 matmul must write fp32 psum).
  - ctx + out_proj in fp16; flipped ctx layout [q_tile(128), 65] with a
    45.0 ones-column producing the softmax denominator as psum col 64.

ACT is the bottleneck engine: 48 exps x ~1.04us ~= 50us of exp stream.
The schedule keeps ACT saturated: packed host-side weight layouts make
every input DMA >= 512B/descriptor (fast head, first exp ~4.5us), the
48 scores units are interleaved with dependency-ordered PE filler work
(projections, flipped-ctx units, transposes, qt0 out tiles), and the
phase order A=(hp0,qt0) B=(1,0) C=(0,1) D=(2,0) E=(1,1) F=(2,1)
staggers the qt0/qt1 context phases so only pair-2-qt1's ctx units,
per-qs transposes, and the four qt1 out tiles remain after the final
exp. Within a phase the exps run hi-major so head hi=0's ctx units
start two exps early. Head k-psum copies ride ACT (idle pre-exp);
steady-state k plane-1 dups ride gpsimd (idle) to unload DVE.
"""

import numpy as np
import ml_dtypes

import concourse.mybir as mybir
from concourse import bacc
from concourse.tile import TileContext
from concourse.bass_utils import run_bass_kernel_spmd

FP = mybir.dt.float32
F16 = mybir.dt.float16
F8 = mybir.dt.float8e4
AF = mybir.ActivationFunctionType
DR = mybir.MatmulPerfMode.DoubleRow

E4NP = ml_dtypes.float8_e4m3

B, T, D = 4, 1024, 768
H, DH = 12, 64
NCORES = 8
HPC = 6           # heads per core
DPC = HPC * DH    # 384 head-dims per core
KC = D // 128     # 6 contraction chunks for d_in
CP = KC // 2      # 3 chunk-pairs for DoubleRow
MC = DPC // 128   # 3 chunks of per-core head dims (= head PAIRS)
NT = T // 512     # 2 free-dim tiles of tokens
TT = T // 128     # 8 partition tiles of tokens

WSCALE = 45.0                       # host scale on Wq/Wk/Wv
EXP_SCALE = 1.0 / (WSCALE * WSCALE * 8.0)  # S_psum = 45q . 45k = 16200*(qk/8)
ONES_VAL = WSCALE                   # denominator column matches the 45*v scale


def emit_mha(tc, xh, xl, wq8, wk8, wv8, wo, ones, ident, out, ctx):
    nc = tc.nc

    singles = ctx.enter_context(tc.tile_pool(name="singles", bufs=1))
    proj_psum = ctx.enter_context(tc.tile_pool(name="proj_psum", bufs=2, space="PSUM"))
    scores_psum = ctx.enter_context(
        tc.tile_pool(name="scores_psum", bufs=2, space="PSUM")
    )
    # ctx psum tiles share the proj pool (scores tiles need 3 banks x 2)
    ctx_psum = proj_psum
    expS_pool = ctx.enter_context(tc.tile_pool(name="expS", bufs=20))
    rcp_pool = ctx.enter_context(tc.tile_pool(name="rcp", bufs=10))
    ctxN_pool = ctx.enter_context(tc.tile_pool(name="ctxN", bufs=18))
    out_pool = ctx.enter_context(tc.tile_pool(name="outsb", bufs=6))

    # ---------------- staged input DMAs ----------------
    # All host tensors are pre-packed so every transfer moves >=512B per
    # descriptor (the cost model charges 2x below 512B). Weight m-chunks
    # are contiguous so the head-critical slices arrive in one descriptor
    # sweep each.
    xh_sb = singles.tile([128, KC, T], F8, name="xh_sb", tag="xh_sb")
    xl_sb = singles.tile([128, KC, T], F8, name="xl_sb", tag="xl_sb")
    wq_sb = singles.tile([128, MC, KC, 2, 128], F8, name="wq_sb", tag="wq_sb")
    wk_sb = singles.tile([128, MC, KC, 128], F8, name="wk_sb", tag="wk_sb")
    wv_sb = singles.tile([128, KC, 2, DPC], F8, name="wv_sb", tag="wv_sb")
    wo_sb = singles.tile([128, MC, D], F16, name="wo_sb", tag="wo_sb")
    ones_sb = singles.tile([128, HPC], F16, name="ones_sb", tag="ones_sb")
    ident_sb = singles.tile([128, 128], F16, name="ident_sb", tag="ident_sb")

    # critical chain for the first scores unit: xh n0, wq m0, wk m0.
    # Alternate issue sequencers so the queue is fed without stalling
    # ACT's sequencer (which must stay clear to decode the first exps).
    nc.sync.dma_start(out=xh_sb[:, :, 0:512], in_=xh.rearrange("(c p) t -> p c t", p=128)[:, :, 0:512])
    nc.scalar.dma_start(out=wq_sb[:, 0], in_=wq8[:, 0])
    nc.sync.dma_start(out=wk_sb[:, 0], in_=wk8[:, 0])
    nc.scalar.dma_start(out=xh_sb[:, :, 512:1024], in_=xh.rearrange("(c p) t -> p c t", p=128)[:, :, 512:1024])
    nc.gpsimd.dma_start(out=ones_sb, in_=ones)
    # non-critical: all on the SP sequencer
    nc.sync.dma_start(out=wq_sb[:, 1], in_=wq8[:, 1])
    nc.sync.dma_start(out=wk_sb[:, 1], in_=wk8[:, 1])
    nc.sync.dma_start(out=wq_sb[:, 2], in_=wq8[:, 2])
    nc.sync.dma_start(out=wk_sb[:, 2], in_=wk8[:, 2])
    nc.sync.dma_start(out=xl_sb[:, :, 0:512], in_=xl.rearrange("(c p) t -> p c t", p=128)[:, :, 0:512])
    nc.sync.dma_start(out=xl_sb[:, :, 512:1024], in_=xl.rearrange("(c p) t -> p c t", p=128)[:, :, 512:1024])
    nc.sync.dma_start(out=wv_sb, in_=wv8)
    nc.sync.dma_start(out=wo_sb, in_=wo.rearrange("(c p) d -> p c d", p=128))
    nc.sync.dma_start(out=ident_sb, in_=ident)

    # warm-up: dummy matmul chain keeps PE busy from ~1.2us until the
    # first inputs land so the cost model's 3us p-state ramp elapses on
    # throwaway work (the ramp clock resets whenever PE goes idle).
    wu_sb = singles.tile([128, 256], F16, name="wu_sb", tag="wu_sb")
    nc.vector.memset(wu_sb, 0.0)
    for _ in range(4):
        ps_wu = proj_psum.tile([128, 512], FP, name="ps_wu", tag="proj")
        nc.tensor.matmul(ps_wu[:, 0:256], lhsT=wu_sb[:, 0:128],
                         rhs=wu_sb[:, 0:256], start=True, stop=True)
        nc.tensor.matmul(ps_wu[:, 256:512], lhsT=wu_sb[:, 0:128],
                         rhs=wu_sb[:, 0:256], start=True, stop=True)

    q8_sb = singles.tile([128, MC, 2, T], F8, name="q8_sb", tag="q8_sb")
    k8_sb = singles.tile([128, MC, 2, T], F8, name="k8_sb", tag="k8_sb")
    ctxT_sb = singles.tile([128, MC, T], F16, name="ctxT_sb", tag="ctxT_sb")

    # v tiles [t_tile, 6 heads x (64 v cols + ones col)]: the 45.0 column
    # makes each head's ctx matmul also produce its softmax denominator.
    v_sb = [singles.tile([128, HPC, DH + 1], F16, name=f"v_sb{i}", tag=f"v_sb{i}")
            for i in range(TT)]

    def ones_fanout():
        for vt in v_sb:
            nc.gpsimd.tensor_copy(vt[:, :, DH : DH + 1], ones_sb)

    def qk_proj(m, n, dsts=("k", "q"), k_on_act=False, pool=None, fine=False):
        # q'/k' chunk m, token block n: psum[dout(128), t(512)]
        pool = pool or proj_psum
        sl = slice(n * 512, (n + 1) * 512)
        for d in dsts:
            ps = pool.tile([128, 512], FP, name="ps_qk", tag="proj")
            if d == "q":
                first = True
                for t in range(2):          # x_hi.w_hi + x_hi.w_lo
                    for cp in range(CP):
                        nc.tensor.matmul(
                            ps,
                            lhsT=wq_sb[:, m, 2 * cp : 2 * cp + 2, t, :],
                            rhs=xh_sb[:, 2 * cp : 2 * cp + 2, sl],
                            start=first,
                            stop=(t == 1 and cp == CP - 1),
                            perf_mode=DR,
                        )
                        first = False
                if fine:
                    # half-width copy+sub pairs so the split first scores
                    # unit can start on q columns 0:256 early
                    for h2 in range(2):
                        hsl = slice(n * 512 + h2 * 256, n * 512 + (h2 + 1) * 256)
                        psl = slice(h2 * 256, (h2 + 1) * 256)
                        nc.vector.tensor_copy(q8_sb[:, m, 0, hsl], ps[:, psl])
                        nc.vector.tensor_sub(q8_sb[:, m, 1, hsl], ps[:, psl],
                                             q8_sb[:, m, 0, hsl])
                else:
                    nc.vector.tensor_copy(q8_sb[:, m, 0, sl], ps)
                    nc.vector.tensor_sub(q8_sb[:, m, 1, sl], ps, q8_sb[:, m, 0, sl])
            else:
                # k: 1-term (w_hi only) -- k is requantized to single fp8
                # for the scores matmul, so w_lo precision is wasted there
                first = True
                for cp in range(CP):
                    nc.tensor.matmul(
                        ps,
                        lhsT=wk_sb[:, m, 2 * cp : 2 * cp + 2, :],
                        rhs=xh_sb[:, 2 * cp : 2 * cp + 2, sl],
                        start=first,
                        stop=(cp == CP - 1),
                        perf_mode=DR,
                    )
                    first = False
                if k_on_act:
                    # head region: ACT is idle pre-exp; both plane copies
                    # run parallel to the q copies on DVE. Split at token
                    # 384 so k-tiles 0-2 (the first scores unit) go first.
                    for ksl, psl in ((slice(0, 384), slice(0, 384)),
                                     (slice(384, 512), slice(384, 512))):
                        nc.scalar.copy(k8_sb[:, m, 0, ksl], ps[:, psl])
                        nc.scalar.copy(k8_sb[:, m, 1, ksl], ps[:, psl])
                else:
                    nc.vector.tensor_copy(k8_sb[:, m, 0, sl], ps)
                    # plane-1 dup from SBUF on gpsimd (idle; DVE relief)
                    nc.gpsimd.tensor_copy(k8_sb[:, m, 1, sl], k8_sb[:, m, 0, sl])

    def v_proj(mts):
        # v': psum[t_tile(128), dh(384)] = 45 * sum_c x[c].T wv[c]
        for mt in mts:
            ps = proj_psum.tile([128, DPC], FP, name="ps_v", tag="proj")
            first = True
            for xt, t in ((xh_sb, 0), (xh_sb, 1), (xl_sb, 0)):
                for cp in range(CP):
                    nc.tensor.matmul(
                        ps,
                        lhsT=xt[:, 2 * cp : 2 * cp + 2, mt * 128 : (mt + 1) * 128],
                        rhs=wv_sb[:, 2 * cp : 2 * cp + 2, t, :],
                        start=first,
                        stop=(xt is xl_sb and cp == CP - 1),
                        perf_mode=DR,
                    )
                    first = False
            nc.vector.tensor_copy(v_sb[mt][:, :, 0:DH], ps)

    # exp tiles per (hp, qt, hi): k-tile groups (0-2, 3-5, 6-7); the wide
    # tiles amortize the ~185ns/inst PSUM/SBUF access overhead on ACT over
    # 1536 columns instead of 1024
    KGRP = ((0, 1, 2), (3, 4, 5), (6, 7))
    exps = {}

    def scores_unit_split(hp, qt, hi, g):
        # head variant: same psum tile, but matmuls and exp run per
        # 256-col q-half so the exp stream starts on half the q copies
        po = 64 * hi
        js = KGRP[g]
        ps = scores_psum.tile([128, len(js), 512], FP, name="ps_s", tag="scores")
        halves = []
        for h2 in range(2):
            for r, j in enumerate(js):
                nc.tensor.matmul(
                    ps[:, r, h2 * 256 : (h2 + 1) * 256],
                    lhsT=k8_sb[po : po + 64, hp, :, j * 128 : (j + 1) * 128],
                    rhs=q8_sb[po : po + 64, hp, :,
                              qt * 512 + h2 * 256 : qt * 512 + (h2 + 1) * 256],
                    start=True,
                    stop=True,
                    perf_mode=DR,
                )
            ex = expS_pool.tile([128, len(js), 256], F16, name="exh", tag="expS")
            nc.scalar.activation(ex, ps[:, :, h2 * 256 : (h2 + 1) * 256],
                                 AF.Exp, scale=EXP_SCALE)
            halves.append(ex)
        exps.setdefault((hp, qt, hi), [None] * 3)[g] = tuple(halves)

    def scores_unit(hp, qt, hi, g):
        po = 64 * hi
        js = KGRP[g]
        ps = scores_psum.tile([128, 512 * len(js)], FP, name="ps_s", tag="scores")
        for r, j in enumerate(js):
            nc.tensor.matmul(
                ps[:, r * 512 : (r + 1) * 512],
                lhsT=k8_sb[po : po + 64, hp, :, j * 128 : (j + 1) * 128],
                rhs=q8_sb[po : po + 64, hp, :, qt * 512 : (qt + 1) * 512],
                start=True,
                stop=True,
                perf_mode=DR,
            )
        ex = expS_pool.tile([128, 512 * len(js)], F16, name="ex", tag="expS")
        nc.scalar.activation(ex, ps, AF.Exp, scale=EXP_SCALE)
        exps.setdefault((hp, qt, hi), [None] * 3)[g] = ex

    cn_store = {}

    def ctx_q(hp, qt, hi, qs):
        # flipped ctx: out[q_tile(128), 65] = sum_kt expST[kt, q].T @ [45v|45]
        ex = exps[(hp, qt, hi)]
        h = 2 * hp + hi
        pc = ctx_psum.tile([128, 65], FP, name="pcq", tag="proj")
        c0 = qs * 128
        for j in range(TT):
            g, r = (j // 3, j % 3) if j < 6 else (2, j - 6)
            if isinstance(ex[g], tuple):
                lhsT = ex[g][qs // 2][:, r, (qs % 2) * 128 : (qs % 2) * 128 + 128]
            else:
                lhsT = ex[g][:, r * 512 + c0 : r * 512 + c0 + 128]
            nc.tensor.matmul(
                pc,
                lhsT=lhsT,
                rhs=v_sb[j][:, h, :],
                start=(j == 0),
                stop=(j == TT - 1),
            )
        rcp = rcp_pool.tile([128, 1], FP, name="rcp", tag="rcp")
        cn = ctxN_pool.tile([128, DH], F16, name="ctxN", tag="ctxN")
        nc.vector.reciprocal(rcp, pc[:, DH : DH + 1])
        nc.vector.tensor_scalar_mul(cn, pc[:, 0:DH], rcp)
        cn_store.setdefault((hp, qt), [[None] * 4 for _ in range(2)])[hi][qs] = cn

    def ctx_t(hp, qt):
        # batched transpose of the pair's eight [q(128), dh(64)] tiles back
        # into ctxT layout via one fp16 psum + a single 2x-mode DVE copy
        cns = cn_store[(hp, qt)]
        pt = proj_psum.tile([128, 512], F16, name="pt", tag="proj")
        for hi in range(2):
            po = 64 * hi
            for qs in range(4):
                nc.tensor.transpose(
                    pt[po : po + 64, qs * 128 : (qs + 1) * 128],
                    cns[hi][qs],
                    ident_sb,
                )
        nc.vector.tensor_copy(ctxT_sb[:, hp, qt * 512 : (qt + 1) * 512], pt)

    def ctx_t_qs(hp, qt, qs):
        # tail variant: per-qs transpose so out tile mt=4qt+qs unblocks
        # right after its own q-slice, not after the whole pair
        cns = cn_store[(hp, qt)]
        pt = proj_psum.tile([128, 128], F16, name="ptq", tag="proj")
        for hi in range(2):
            po = 64 * hi
            nc.tensor.transpose(pt[po : po + 64, :], cns[hi][qs], ident_sb)
        nc.vector.tensor_copy(
            ctxT_sb[:, hp, qt * 512 + qs * 128 : qt * 512 + (qs + 1) * 128], pt)

    # paired output staging: tiles (2i, 2i+1) share one [128, 2, 768] SBUF
    # buffer and leave in ONE dma (each dma_start costs ~630ns of exclusive
    # HWDGE issue time, so halving the count shortens the tail directly)
    osb_pairs = [singles.tile([128, 2, D], F16, name=f"osb{i}", tag=f"osb{i}")
                 for i in range(4)]

    def out_proj(mt, tail=False):
        # out[t_tile(128), dout(768)] = sum_c ctxT16[c].T @ wo16[c] in two
        # 384-col halves. Tail tiles borrow the (dead by then) scores psum
        # pool for the second half and put that copy on ACT (idle post-exp).
        osb = osb_pairs[mt // 2]
        for n2 in range(2):
            pool = scores_psum if (tail and n2 == 1) else proj_psum
            ps = pool.tile([128, 384], FP, name="ps_o",
                           tag="proj" if pool is proj_psum else "scores")
            for c in range(MC):
                nc.tensor.matmul(
                    ps,
                    lhsT=ctxT_sb[:, c, mt * 128 : (mt + 1) * 128],
                    rhs=wo_sb[:, c, n2 * 384 : (n2 + 1) * 384],
                    start=(c == 0),
                    stop=(c == MC - 1),
                )
            csl = slice(n2 * 384, (n2 + 1) * 384)
            if tail and n2 == 0:
                # ACT is idle once the exp stream drains; splitting the two
                # halves across ACT/DVE halves the copy latency per tile
                nc.scalar.copy(osb[:, mt % 2, csl], ps)
            else:
                nc.vector.tensor_copy(osb[:, mt % 2, csl], ps)
        orr = out.rearrange("(b p) d -> p b d", p=128)
        if mt >= 6:
            # last two tiles leave individually so mt6's transfer overlaps
            # mt7's compute and the final DMA is a short one
            nc.sync.dma_start(out=orr[:, mt : mt + 1, :],
                              in_=osb[:, mt % 2 : mt % 2 + 1, :])
        elif mt % 2 == 1:
            # pair complete: one DMA for rows (mt-1)*128 .. (mt+1)*128
            nc.sync.dma_start(out=orr[:, mt - 1 : mt + 1, :], in_=osb)

    # ---------------- schedule ----------------
    # Phase order staggers qt so ctx/transpose/out work spreads out:
    #   A=(0,0) B=(1,0) C=(0,1) D=(2,0) E=(1,1) F=(2,1)
    # Units within a phase run hi-major (hi0 g0..3, hi1 g0..3) so the
    # hi0 ctx units become available four exps before the phase ends.
    def phase_units(hp, qt):
        return [(hp, qt, hi, g) for hi in range(2) for g in range(3)]

    # head: critical qk m0/n0 (k copies on ACT -- idle pre-exp), phase-A
    # units interleaved with the remaining projection groups
    # head: unit (hi, g=0) needs only k/q n0; the n1 groups have two
    # exps (~3us) of slack before unit g=1 (k-tiles 3-5 span both halves)
    qk_proj(0, 0, k_on_act=True, fine=True)
    scores_unit_split(0, 0, 0, 0)
    qk_proj(0, 1, dsts=("k",))
    scores_unit(0, 0, 1, 0)
    qk_proj(0, 1, dsts=("q",))
    scores_unit(0, 0, 0, 1)
    qk_proj(1, 0, dsts=("k",))
    scores_unit(0, 0, 0, 2)
    qk_proj(1, 0, dsts=("q",))
    scores_unit(0, 0, 1, 1)
    ones_fanout()
    scores_unit(0, 0, 1, 2)

    stream = (phase_units(1, 0) + phase_units(0, 1) + phase_units(2, 0)
              + phase_units(1, 1) + phase_units(2, 1))

    # filler groups in dependency order, consumed one per scores unit
    # (each scores unit is ~0.21us PE vs a ~1.04us exp, so PE has ~0.8us
    # of filler headroom per unit)
    fillers = [
        # during B: remaining projections, v tiles
        [lambda: qk_proj(1, 1)],
        [lambda: qk_proj(2, 0)],
        [lambda: qk_proj(2, 1)],
        [lambda: v_proj([0])],
        [lambda: v_proj([1])],
        [lambda: v_proj([2])],
        # during C: rest of v, ctxA
        [lambda: v_proj([3])],
        [lambda: v_proj([4])],
        [lambda: v_proj([5])],
        [lambda: v_proj([6])],
        [lambda: v_proj([7])],
        [lambda qs=qs: ctx_q(0, 0, 0, qs) for qs in range(4)],
        # during D: ctxA hi1, ctxB, ctxC hi0
        [lambda qs=qs: ctx_q(0, 0, 1, qs) for qs in range(4)] + [lambda: ctx_t(0, 0)],
        [lambda qs=qs: ctx_q(1, 0, 0, qs) for qs in range(4)],
        [lambda qs=qs: ctx_q(1, 0, 1, qs) for qs in range(4)] + [lambda: ctx_t(1, 0)],
        [lambda qs=qs: ctx_q(0, 1, 0, qs) for qs in range(4)],
        [lambda qs=qs: ctx_q(0, 1, 1, qs) for qs in range(4)] + [lambda: ctx_t(0, 1)],
        [],
        # during E: ctxD + out qt0
        [lambda qs=qs: ctx_q(2, 0, 0, qs) for qs in range(4)],
        [lambda qs=qs: ctx_q(2, 0, 1, qs) for qs in range(4)] + [lambda: ctx_t(2, 0)],
        [lambda: out_proj(0)],
        [lambda: out_proj(1)],
        # during F: out qt0 tail, ctxE, ctxF hi0
        [lambda: out_proj(2)],
        [lambda: out_proj(3)],
        [lambda qs=qs: ctx_q(1, 1, 0, qs) for qs in range(4)],
        [lambda qs=qs: ctx_q(1, 1, 1, qs) for qs in range(4)] + [lambda: ctx_t(1, 1)],
        [lambda qs=qs: ctx_q(2, 1, 0, qs) for qs in range(4)],
    ]

    fi = iter(fillers)
    for u in stream:
        scores_unit(*u)
        grp = next(fi, None)
        if grp is not None:
            for f in grp:
                f()
    for grp in fi:
        for f in grp:
            f()

    # tail: after F's last exp only pair-2-qt1's hi1 ctx units, per-qs
    # transposes, and the qt1 out tiles remain. Emission is software-
    # pipelined so PE never waits on a DVE round-trip: while out tile qs
    # waits on its ctxT copy, PE runs ctx/transpose work for qs+1.
    ctx_q(2, 1, 1, 0)
    ctx_q(2, 1, 1, 1)
    ctx_t_qs(2, 1, 0)
    ctx_q(2, 1, 1, 2)
    ctx_t_qs(2, 1, 1)
    out_proj(4, tail=True)
    ctx_q(2, 1, 1, 3)
    ctx_t_qs(2, 1, 2)
    out_proj(5, tail=True)
    ctx_t_qs(2, 1, 3)
    out_proj(6, tail=True)
    out_proj(7, tail=True)


_PROGRAM = None


def build_program():
    global _PROGRAM
    if _PROGRAM is not None:
        return _PROGRAM
    nc = bacc.Bacc("TRN2", target_bir_lowering=False, debug=False, num_devices=NCORES)
    xh = nc.dram_tensor("xh", (D, T), F8, kind="ExternalInput").ap()
    xl = nc.dram_tensor("xl", (D, T), F8, kind="ExternalInput").ap()
    wq8 = nc.dram_tensor("wq8", (128, MC, KC, 2, 128), F8, kind="ExternalInput").ap()
    wk8 = nc.dram_tensor("wk8", (128, MC, KC, 128), F8, kind="ExternalInput").ap()
    wv8 = nc.dram_tensor("wv8", (128, KC, 2, DPC), F8, kind="ExternalInput").ap()
    wo = nc.dram_tensor("wo", (DPC, D), F16, kind="ExternalInput").ap()
    ones = nc.dram_tensor("ones", (128, HPC), F16, kind="ExternalInput").ap()
    ident = nc.dram_tensor("ident", (128, 128), F16, kind="ExternalInput").ap()
    out = nc.dram_tensor("out", (T, D), F16, kind="ExternalOutput").ap()
    from contextlib import ExitStack

    with TileContext(nc) as tc, ExitStack() as st:
        emit_mha(tc, xh, xl, wq8, wk8, wv8, wo, ones, ident, out, st)
    nc.compile()
    _PROGRAM = nc
    return nc


def _split8(a):
    hi = np.clip(a, -240.0, 240.0).astype(E4NP)
    lo = np.clip(a - hi.astype(np.float32), -240.0, 240.0).astype(E4NP)
    return hi, lo


def _pack_qk(w, both_terms=True):
    # w: [DPC, D] torch-layout slice -> packed [128, MC, KC, (2,) 128] with
    # m-chunks contiguous per partition (one >=512B-elem DMA per m-chunk)
    wt = w.T.astype(np.float32) * WSCALE          # [D, DPC]
    hi, lo = _split8(wt)
    hi = hi.reshape(KC, 128, MC, 128)
    if not both_terms:
        return np.ascontiguousarray(hi.transpose(1, 2, 0, 3))
    lo = lo.reshape(KC, 128, MC, 128)
    arr = np.stack([hi, lo], axis=0)              # [2, c, p, m, j]
    return np.ascontiguousarray(arr.transpose(2, 3, 1, 0, 4))


def _pack_v(w):
    wt = w.T.astype(np.float32) * WSCALE
    hi, lo = _split8(wt)
    arr = np.stack([hi.reshape(KC, 128, DPC), lo.reshape(KC, 128, DPC)], axis=0)
    return np.ascontiguousarray(arr.transpose(2, 1, 0, 3))  # [p, c, 2, d]


def make_in_maps(x, Wq, Wk, Wv, Wo):
    x = np.asarray(x, dtype=np.float32)
    ones = np.full((128, HPC), ONES_VAL, np.float16)
    ident = np.eye(128, dtype=np.float16)
    xs = [_split8(x[b].T) for b in range(B)]
    xs = [(np.ascontiguousarray(h), np.ascontiguousarray(l)) for h, l in xs]
    in_maps = []
    for core in range(NCORES):
        b, hh = core // 2, core % 2
        sl = slice(hh * DPC, (hh + 1) * DPC)
        in_maps.append(
            {
                "xh": xs[b][0],
                "xl": xs[b][1],
                "wq8": _pack_qk(np.asarray(Wq)[sl]),
                "wk8": _pack_qk(np.asarray(Wk)[sl], both_terms=False),
                "wv8": _pack_v(np.asarray(Wv)[sl]),
                "wo": np.ascontiguousarray(np.asarray(Wo)[:, sl].T.astype(np.float16)),
                "ones": ones,
                "ident": ident,
            }
        )
    return in_maps


def kernel(x, Wq, Wk, Wv, Wo, bo):
    nc = build_program()
    in_maps = make_in_maps(x, Wq, Wk, Wv, Wo)
    res = run_bass_kernel_spmd(nc, in_maps, core_ids=list(range(NCORES)))
    bo = np.asarray(bo, dtype=np.float32)
    out = np.empty((B, T, D), dtype=np.float32)
    for b in range(B):
        out[b] = (res.results[2 * b]["out"].astype(np.float32)
                  + res.results[2 * b + 1]["out"].astype(np.float32) + bo)
    return out


# revision 17
# speedup vs baseline: 1.0095x; 1.0095x over previous
"""MultiHeadAttention Trainium2 kernel (8 NeuronCores, SPMD).

Reference computation (B=4, T=1024, D=768, H=12, Dh=64):
    q = x @ Wq.T ; k = x @ Wk.T ; v = x @ Wv.T       (per-head reshape)
    attn = softmax((q @ k.T) / 8)
    out = (attn @ v) @ Wo.T + bo
Sharding: 8 cores = 4 batches x 2 head-halves (6 heads each); host sums
the two partials per batch and adds the bias.

fp8 DoubleRow everywhere on the PE (0.5 cycles/output-column):
  - q: (hi, lo) compensation pair from a 2-term DR projection.
  - k: 1-term (w_hi only) DR projection, requantized to a duplicated
    single-fp8 pair (k feeds the scores matmul as fp8 anyway).
  - v: 3-term compensated projection (accuracy matters directly).
  - scores: DR with moving (q_hi, q_lo) pair vs stationary dup-k pair,
    fp32 PSUM tiles [128, 1024] (TRN2 matmul must write fp32 psum).
  - ctx + out_proj in fp16; flipped ctx layout [q_tile(128), 65] with a
    45.0 ones-column producing the softmax denominator as psum col 64.

ACT is the bottleneck engine: 48 exps x ~1.04us ~= 50us of exp stream.
The schedule keeps ACT saturated: packed host-side weight layouts make
every input DMA >= 512B/descriptor (fast head, first exp ~4.5us), the
48 scores units are interleaved with dependency-ordered PE filler work
(projections, flipped-ctx units, transposes, qt0 out tiles), and the
phase order A=(hp0,qt0) B=(1,0) C=(0,1) D=(2,0) E=(1,1) F=(2,1)
staggers the qt0/qt1 context phases so only pair-2-qt1's ctx units,
per-qs transposes, and the four qt1 out tiles remain after the final
exp. Within a phase the exps run hi-major so head hi=0's ctx units
start two exps early. Head k-psum copies ride ACT (idle pre-exp);
steady-state k plane-1 dups ride gpsimd (idle) to unload DVE.
"""

import numpy as np
import ml_dtypes

import concourse.mybir as mybir
from concourse import bacc
from concourse.tile import TileContext
from concourse.bass_utils import run_bass_kernel_spmd

FP = mybir.dt.float32
F16 = mybir.dt.float16
F8 = mybir.dt.float8e4
AF = mybir.ActivationFunctionType
DR = mybir.MatmulPerfMode.DoubleRow

E4NP = ml_dtypes.float8_e4m3

B, T, D = 4, 1024, 768
H, DH = 12, 64
NCORES = 8
HPC = 6           # heads per core
DPC = HPC * DH    # 384 head-dims per core
KC = D // 128     # 6 contraction chunks for d_in
CP = KC // 2      # 3 chunk-pairs for DoubleRow
MC = DPC // 128   # 3 chunks of per-core head dims (= head PAIRS)
NT = T // 512     # 2 free-dim tiles of tokens
TT = T // 128     # 8 partition tiles of tokens

WSCALE = 45.0                       # host scale on Wq/Wk/Wv
EXP_SCALE = 1.0 / (WSCALE * WSCALE * 8.0)  # S_psum = 45q . 45k = 16200*(qk/8)
ONES_VAL = WSCALE                   # denominator column matches the 45*v scale


def emit_mha(tc, xh, xl, wq8, wk8, wv8, wo, ones, ident, out, ctx):
    nc = tc.nc

    singles = ctx.enter_context(tc.tile_pool(name="singles", bufs=1))
    proj_psum = ctx.enter_context(tc.tile_pool(name="proj_psum", bufs=2, space="PSUM"))
    scores_psum = ctx.enter_context(
        tc.tile_pool(name="scores_psum", bufs=2, space="PSUM")
    )
    # ctx psum tiles share the proj pool (scores tiles need 3 banks x 2)
    ctx_psum = proj_psum
    expS_pool = ctx.enter_context(tc.tile_pool(name="expS", bufs=20))
    rcp_pool = ctx.enter_context(tc.tile_pool(name="rcp", bufs=10))
    ctxN_pool = ctx.enter_context(tc.tile_pool(name="ctxN", bufs=18))
    out_pool = ctx.enter_context(tc.tile_pool(name="outsb", bufs=6))

    # ---------------- staged input DMAs ----------------
    # All host tensors are pre-packed so every transfer moves >=512B per
    # descriptor (the cost model charges 2x below 512B). Weight m-chunks
    # are contiguous so the head-critical slices arrive in one descriptor
    # sweep each.
    xh_sb = singles.tile([128, KC, T], F8, name="xh_sb", tag="xh_sb")
    xl_sb = singles.tile([128, KC, T], F8, name="xl_sb", tag="xl_sb")
    wq_sb = singles.tile([128, MC, KC, 2, 128], F8, name="wq_sb", tag="wq_sb")
    wk_sb = singles.tile([128, MC, KC, 128], F8, name="wk_sb", tag="wk_sb")
    wv_sb = singles.tile([128, KC, 2, DPC], F8, name="wv_sb", tag="wv_sb")
    wo_sb = singles.tile([128, MC, D], F16, name="wo_sb", tag="wo_sb")
    ones_sb = singles.tile([128, HPC], F16, name="ones_sb", tag="ones_sb")
    ident_sb = singles.tile([128, 128], F16, name="ident_sb", tag="ident_sb")

    # critical chain for the first scores unit: xh n0, wq m0, wk m0.
    # Alternate issue sequencers so the queue is fed without stalling
    # ACT's sequencer (which must stay clear to decode the first exps).
    nc.sync.dma_start(out=xh_sb[:, :, 0:512], in_=xh.rearrange("(c p) t -> p c t", p=128)[:, :, 0:512])
    nc.scalar.dma_start(out=wq_sb[:, 0], in_=wq8[:, 0])
    nc.sync.dma_start(out=wk_sb[:, 0], in_=wk8[:, 0])
    nc.scalar.dma_start(out=xh_sb[:, :, 512:1024], in_=xh.rearrange("(c p) t -> p c t", p=128)[:, :, 512:1024])
    nc.gpsimd.dma_start(out=ones_sb, in_=ones)
    # non-critical: all on the SP sequencer
    nc.sync.dma_start(out=wq_sb[:, 1], in_=wq8[:, 1])
    nc.sync.dma_start(out=wk_sb[:, 1], in_=wk8[:, 1])
    nc.sync.dma_start(out=wq_sb[:, 2], in_=wq8[:, 2])
    nc.sync.dma_start(out=wk_sb[:, 2], in_=wk8[:, 2])
    nc.sync.dma_start(out=xl_sb[:, :, 0:512], in_=xl.rearrange("(c p) t -> p c t", p=128)[:, :, 0:512])
    nc.sync.dma_start(out=xl_sb[:, :, 512:1024], in_=xl.rearrange("(c p) t -> p c t", p=128)[:, :, 512:1024])
    nc.sync.dma_start(out=wv_sb, in_=wv8)
    nc.sync.dma_start(out=wo_sb, in_=wo.rearrange("(c p) d -> p c d", p=128))
    nc.sync.dma_start(out=ident_sb, in_=ident)

    # warm-up: dummy matmul chain keeps PE busy from ~1.2us until the
    # first inputs land so the cost model's 3us p-state ramp elapses on
    # throwaway work (the ramp clock resets whenever PE goes idle).
    wu_sb = singles.tile([128, 256], F16, name="wu_sb", tag="wu_sb")
    nc.vector.memset(wu_sb, 0.0)
    for _ in range(4):
        ps_wu = proj_psum.tile([128, 512], FP, name="ps_wu", tag="proj")
        nc.tensor.matmul(ps_wu[:, 0:256], lhsT=wu_sb[:, 0:128],
                         rhs=wu_sb[:, 0:256], start=True, stop=True)
        nc.tensor.matmul(ps_wu[:, 256:512], lhsT=wu_sb[:, 0:128],
                         rhs=wu_sb[:, 0:256], start=True, stop=True)

    q8_sb = singles.tile([128, MC, 2, T], F8, name="q8_sb", tag="q8_sb")
    k8_sb = singles.tile([128, MC, 2, T], F8, name="k8_sb", tag="k8_sb")
    ctxT_sb = singles.tile([128, MC, T], F16, name="ctxT_sb", tag="ctxT_sb")

    # v tiles [t_tile, 6 heads x (64 v cols + ones col)]: the 45.0 column
    # makes each head's ctx matmul also produce its softmax denominator.
    v_sb = [singles.tile([128, HPC, DH + 1], F16, name=f"v_sb{i}", tag=f"v_sb{i}")
            for i in range(TT)]

    def ones_fanout():
        for vt in v_sb:
            nc.gpsimd.tensor_copy(vt[:, :, DH : DH + 1], ones_sb)

    def qk_proj(m, n, dsts=("k", "q"), k_on_act=False, pool=None, fine=False):
        # q'/k' chunk m, token block n: psum[dout(128), t(512)]
        pool = pool or proj_psum
        sl = slice(n * 512, (n + 1) * 512)
        for d in dsts:
            ps = pool.tile([128, 512], FP, name="ps_qk", tag="proj")
            if d == "q":
                first = True
                for t in range(2):          # x_hi.w_hi + x_hi.w_lo
                    for cp in range(CP):
                        nc.tensor.matmul(
                            ps,
                            lhsT=wq_sb[:, m, 2 * cp : 2 * cp + 2, t, :],
                            rhs=xh_sb[:, 2 * cp : 2 * cp + 2, sl],
                            start=first,
                            stop=(t == 1 and cp == CP - 1),
                            perf_mode=DR,
                        )
                        first = False
                if fine:
                    # half-width copy+sub pairs so the split first scores
                    # unit can start on q columns 0:256 early
                    for h2 in range(2):
                        hsl = slice(n * 512 + h2 * 256, n * 512 + (h2 + 1) * 256)
                        psl = slice(h2 * 256, (h2 + 1) * 256)
                        nc.vector.tensor_copy(q8_sb[:, m, 0, hsl], ps[:, psl])
                        nc.vector.tensor_sub(q8_sb[:, m, 1, hsl], ps[:, psl],
                                             q8_sb[:, m, 0, hsl])
                else:
                    nc.vector.tensor_copy(q8_sb[:, m, 0, sl], ps)
                    nc.vector.tensor_sub(q8_sb[:, m, 1, sl], ps, q8_sb[:, m, 0, sl])
            else:
                # k: 1-term (w_hi only) -- k is requantized to single fp8
                # for the scores matmul, so w_lo precision is wasted there
                first = True
                for cp in range(CP):
                    nc.tensor.matmul(
                        ps,
                        lhsT=wk_sb[:, m, 2 * cp : 2 * cp + 2, :],
                        rhs=xh_sb[:, 2 * cp : 2 * cp + 2, sl],
                        start=first,
                        stop=(cp == CP - 1),
                        perf_mode=DR,
                    )
                    first = False
                if k_on_act:
                    # head region: ACT is idle pre-exp; both plane copies
                    # run parallel to the q copies on DVE. Split at token
                    # 384 so k-tiles 0-2 (the first scores unit) go first.
                    for ksl, psl in ((slice(0, 384), slice(0, 384)),
                                     (slice(384, 512), slice(384, 512))):
                        nc.scalar.copy(k8_sb[:, m, 0, ksl], ps[:, psl])
                        nc.scalar.copy(k8_sb[:, m, 1, ksl], ps[:, psl])
                else:
                    nc.vector.tensor_copy(k8_sb[:, m, 0, sl], ps)
                    # plane-1 dup from SBUF on gpsimd (idle; DVE relief)
                    nc.gpsimd.tensor_copy(k8_sb[:, m, 1, sl], k8_sb[:, m, 0, sl])

    def v_proj(mts):
        # v': psum[t_tile(128), dh(384)] = 45 * sum_c x[c].T wv[c]
        for mt in mts:
            ps = proj_psum.tile([128, DPC], FP, name="ps_v", tag="proj")
            first = True
            for xt, t in ((xh_sb, 0), (xh_sb, 1), (xl_sb, 0)):
                for cp in range(CP):
                    nc.tensor.matmul(
                        ps,
                        lhsT=xt[:, 2 * cp : 2 * cp + 2, mt * 128 : (mt + 1) * 128],
                        rhs=wv_sb[:, 2 * cp : 2 * cp + 2, t, :],
                        start=first,
                        stop=(xt is xl_sb and cp == CP - 1),
                        perf_mode=DR,
                    )
                    first = False
            nc.vector.tensor_copy(v_sb[mt][:, :, 0:DH], ps)

    # exp tiles per (hp, qt, hi): k-tile groups (0-2, 3-5, 6-7); the wide
    # tiles amortize the ~185ns/inst PSUM/SBUF access overhead on ACT over
    # 1536 columns instead of 1024
    KGRP = ((0, 1, 2), (3, 4, 5), (6, 7))
    exps = {}

    def scores_unit_split(hp, qt, hi, g):
        # head variant: same psum tile, but matmuls and exp run per
        # 256-col q-half so the exp stream starts on half the q copies
        po = 64 * hi
        js = KGRP[g]
        ps = scores_psum.tile([128, len(js), 512], FP, name="ps_s", tag="scores")
        halves = []
        for h2 in range(2):
            for r, j in enumerate(js):
                nc.tensor.matmul(
                    ps[:, r, h2 * 256 : (h2 + 1) * 256],
                    lhsT=k8_sb[po : po + 64, hp, :, j * 128 : (j + 1) * 128],
                    rhs=q8_sb[po : po + 64, hp, :,
                              qt * 512 + h2 * 256 : qt * 512 + (h2 + 1) * 256],
                    start=True,
                    stop=True,
                    perf_mode=DR,
                )
            ex = expS_pool.tile([128, len(js), 256], F16, name="exh", tag="expS")
            nc.scalar.activation(ex, ps[:, :, h2 * 256 : (h2 + 1) * 256],
                                 AF.Exp, scale=EXP_SCALE)
            halves.append(ex)
        exps.setdefault((hp, qt, hi), [None] * 3)[g] = tuple(halves)

    def scores_unit(hp, qt, hi, g):
        po = 64 * hi
        js = KGRP[g]
        ps = scores_psum.tile([128, 512 * len(js)], FP, name="ps_s", tag="scores")
        for r, j in enumerate(js):
            nc.tensor.matmul(
                ps[:, r * 512 : (r + 1) * 512],
                lhsT=k8_sb[po : po + 64, hp, :, j * 128 : (j + 1) * 128],
                rhs=q8_sb[po : po + 64, hp, :, qt * 512 : (qt + 1) * 512],
                start=True,
                stop=True,
                perf_mode=DR,
            )
        ex = expS_pool.tile([128, 512 * len(js)], F16, name="ex", tag="expS")
        nc.scalar.activation(ex, ps, AF.Exp, scale=EXP_SCALE)
        exps.setdefault((hp, qt, hi), [None] * 3)[g] = ex

    cn_store = {}

    def ctx_q(hp, qt, hi, qs):
        # flipped ctx: out[q_tile(128), 65] = sum_kt expST[kt, q].T @ [45v|45]
        ex = exps[(hp, qt, hi)]
        h = 2 * hp + hi
        pc = ctx_psum.tile([128, 65], FP, name="pcq", tag="proj")
        c0 = qs * 128
        for j in range(TT):
            g, r = (j // 3, j % 3) if j < 6 else (2, j - 6)
            if isinstance(ex[g], tuple):
                lhsT = ex[g][qs // 2][:, r, (qs % 2) * 128 : (qs % 2) * 128 + 128]
            else:
                lhsT = ex[g][:, r * 512 + c0 : r * 512 + c0 + 128]
            nc.tensor.matmul(
                pc,
                lhsT=lhsT,
                rhs=v_sb[j][:, h, :],
                start=(j == 0),
                stop=(j == TT - 1),
            )
        rcp = rcp_pool.tile([128, 1], FP, name="rcp", tag="rcp")
        cn = ctxN_pool.tile([128, DH], F16, name="ctxN", tag="ctxN")
        nc.vector.reciprocal(rcp, pc[:, DH : DH + 1])
        nc.vector.tensor_scalar_mul(cn, pc[:, 0:DH], rcp)
        cn_store.setdefault((hp, qt), [[None] * 4 for _ in range(2)])[hi][qs] = cn

    def ctx_t(hp, qt):
        # batched transpose of the pair's eight [q(128), dh(64)] tiles back
        # into ctxT layout via one fp16 psum + a single 2x-mode DVE copy
        cns = cn_store[(hp, qt)]
        pt = proj_psum.tile([128, 512], F16, name="pt", tag="proj")
        for hi in range(2):
            po = 64 * hi
            for qs in range(4):
                nc.tensor.transpose(
                    pt[po : po + 64, qs * 128 : (qs + 1) * 128],
                    cns[hi][qs],
                    ident_sb,
                )
        nc.vector.tensor_copy(ctxT_sb[:, hp, qt * 512 : (qt + 1) * 512], pt)

    def ctx_t_qs(hp, qt, qs):
        # tail variant: per-qs transpose so out tile mt=4qt+qs unblocks
        # right after its own q-slice, not after the whole pair
        cns = cn_store[(hp, qt)]
        pt = proj_psum.tile([128, 128], F16, name="ptq", tag="proj")
        for hi in range(2):
            po = 64 * hi
            nc.tensor.transpose(pt[po : po + 64, :], cns[hi][qs], ident_sb)
        nc.vector.tensor_copy(
            ctxT_sb[:, hp, qt * 512 + qs * 128 : qt * 512 + (qs + 1) * 128], pt)

    # paired output staging: tiles (2i, 2i+1) share one [128, 2, 768] SBUF
    # buffer and leave in ONE dma (each dma_start costs ~630ns of exclusive
    # HWDGE issue time, so halving the count shortens the tail directly)
    osb_pairs = [singles.tile([128, 2, D], F16, name=f"osb{i}", tag=f"osb{i}")
                 for i in range(4)]

    def out_proj(mt, tail=False, n2s=(0, 1)):
        # out[t_tile(128), dout(768)] = sum_c ctxT16[c].T @ wo16[c] in two
        # 384-col halves. Tail tiles borrow the (dead by then) scores psum
        # pool for the second half and put that copy on ACT (idle post-exp).
        osb = osb_pairs[mt // 2]
        for n2 in n2s:
            pool = scores_psum if (tail and n2 == 1) else proj_psum
            ps = pool.tile([128, 384], FP, name="ps_o",
                           tag="proj" if pool is proj_psum else "scores")
            for c in range(MC):
                nc.tensor.matmul(
                    ps,
                    lhsT=ctxT_sb[:, c, mt * 128 : (mt + 1) * 128],
                    rhs=wo_sb[:, c, n2 * 384 : (n2 + 1) * 384],
                    start=(c == 0),
                    stop=(c == MC - 1),
                )
            csl = slice(n2 * 384, (n2 + 1) * 384)
            if tail and n2 == 0:
                # ACT is idle once the exp stream drains; splitting the two
                # halves across ACT/DVE halves the copy latency per tile
                nc.scalar.copy(osb[:, mt % 2, csl], ps)
            else:
                nc.vector.tensor_copy(osb[:, mt % 2, csl], ps)
        if 1 not in n2s:
            return
        orr = out.rearrange("(b p) d -> p b d", p=128)
        if mt >= 6:
            # last two tiles leave individually so mt6's transfer overlaps
            # mt7's compute and the final DMA is a short one
            nc.sync.dma_start(out=orr[:, mt : mt + 1, :],
                              in_=osb[:, mt % 2 : mt % 2 + 1, :])
        elif mt % 2 == 1:
            # pair complete: one DMA for rows (mt-1)*128 .. (mt+1)*128
            nc.sync.dma_start(out=orr[:, mt - 1 : mt + 1, :], in_=osb)

    # ---------------- schedule ----------------
    # Phase order staggers qt so ctx/transpose/out work spreads out:
    #   A=(0,0) B=(1,0) C=(0,1) D=(2,0) E=(1,1) F=(2,1)
    # Units within a phase run hi-major (hi0 g0..3, hi1 g0..3) so the
    # hi0 ctx units become available four exps before the phase ends.
    def phase_units(hp, qt):
        return [(hp, qt, hi, g) for hi in range(2) for g in range(3)]

    # head: critical qk m0/n0 (k copies on ACT -- idle pre-exp), phase-A
    # units interleaved with the remaining projection groups
    # head: unit (hi, g=0) needs only k/q n0; the n1 groups have two
    # exps (~3us) of slack before unit g=1 (k-tiles 3-5 span both halves)
    qk_proj(0, 0, k_on_act=True, fine=True)
    scores_unit_split(0, 0, 0, 0)
    qk_proj(0, 1, dsts=("k",))
    scores_unit(0, 0, 1, 0)
    qk_proj(0, 1, dsts=("q",))
    scores_unit(0, 0, 0, 1)
    qk_proj(1, 0, dsts=("k",))
    scores_unit(0, 0, 0, 2)
    qk_proj(1, 0, dsts=("q",))
    scores_unit(0, 0, 1, 1)
    ones_fanout()
    scores_unit(0, 0, 1, 2)

    stream = (phase_units(1, 0) + phase_units(0, 1) + phase_units(2, 0)
              + phase_units(1, 1) + phase_units(2, 1))

    # filler groups in dependency order, consumed one per scores unit
    # (each scores unit is ~0.21us PE vs a ~1.04us exp, so PE has ~0.8us
    # of filler headroom per unit)
    fillers = [
        # during B: remaining projections, v tiles
        [lambda: qk_proj(1, 1)],
        [lambda: qk_proj(2, 0)],
        [lambda: qk_proj(2, 1)],
        [lambda: v_proj([0])],
        [lambda: v_proj([1])],
        [lambda: v_proj([2])],
        # during C: rest of v, ctxA
        [lambda: v_proj([3])],
        [lambda: v_proj([4])],
        [lambda: v_proj([5])],
        [lambda: v_proj([6])],
        [lambda: v_proj([7])],
        [lambda qs=qs: ctx_q(0, 0, 0, qs) for qs in range(4)],
        # during D: ctxA hi1, ctxB, ctxC
        [lambda qs=qs: ctx_q(0, 0, 1, qs) for qs in range(4)] + [lambda: ctx_t(0, 0)],
        [lambda qs=qs: ctx_q(1, 0, 0, qs) for qs in range(4)],
        [lambda qs=qs: ctx_q(1, 0, 1, qs) for qs in range(4)] + [lambda: ctx_t(1, 0)],
        [lambda qs=qs: ctx_q(0, 1, 0, qs) for qs in range(4)],
        [lambda qs=qs: ctx_q(0, 1, 1, qs) for qs in range(4)] + [lambda: ctx_t(0, 1)],
        [],
        # during E: ctxD, out qt0 (half-tile groups: one psum+copy per
        # slot keeps the shared proj/ctx pool from head-of-line blocking
        # the scores units)
        [lambda qs=qs: ctx_q(2, 0, 0, qs) for qs in range(4)],
        [lambda qs=qs: ctx_q(2, 0, 1, qs) for qs in range(4)] + [lambda: ctx_t(2, 0)],
        [lambda: out_proj(0, n2s=(0,))],
        [lambda: out_proj(0, n2s=(1,))],
        [lambda: out_proj(1, n2s=(0,))],
        [lambda: out_proj(1, n2s=(1,))],
        # during F: ctxE, out qt0 tail, ctxF hi0
        [lambda qs=qs: ctx_q(1, 1, 0, qs) for qs in range(4)],
        [lambda qs=qs: ctx_q(1, 1, 1, qs) for qs in range(4)] + [lambda: ctx_t(1, 1)],
        [lambda: out_proj(2, n2s=(0,)), lambda: out_proj(2, n2s=(1,))],
        [lambda qs=qs: ctx_q(2, 1, 0, qs) for qs in range(4)],
        [lambda: out_proj(3, n2s=(0,))],
        [lambda: out_proj(3, n2s=(1,))],
    ]

    fi = iter(fillers)
    for u in stream:
        scores_unit(*u)
        grp = next(fi, None)
        if grp is not None:
            for f in grp:
                f()
    for grp in fi:
        for f in grp:
            f()

    # tail: after F's last exp only pair-2-qt1's hi1 ctx units, per-qs
    # transposes, and the qt1 out tiles remain. Emission is software-
    # pipelined so PE never waits on a DVE round-trip: while out tile qs
    # waits on its ctxT copy, PE runs ctx/transpose work for qs+1.
    ctx_q(2, 1, 1, 0)
    ctx_q(2, 1, 1, 1)
    ctx_t_qs(2, 1, 0)
    ctx_q(2, 1, 1, 2)
    ctx_t_qs(2, 1, 1)
    out_proj(4, tail=True)
    ctx_q(2, 1, 1, 3)
    ctx_t_qs(2, 1, 2)
    out_proj(5, tail=True)
    ctx_t_qs(2, 1, 3)
    out_proj(6, tail=True)
    out_proj(7, tail=True)


_PROGRAM = None


def build_program():
    global _PROGRAM
    if _PROGRAM is not None:
        return _PROGRAM
    nc = bacc.Bacc("TRN2", target_bir_lowering=False, debug=False, num_devices=NCORES)
    xh = nc.dram_tensor("xh", (D, T), F8, kind="ExternalInput").ap()
    xl = nc.dram_tensor("xl", (D, T), F8, kind="ExternalInput").ap()
    wq8 = nc.dram_tensor("wq8", (128, MC, KC, 2, 128), F8, kind="ExternalInput").ap()
    wk8 = nc.dram_tensor("wk8", (128, MC, KC, 128), F8, kind="ExternalInput").ap()
    wv8 = nc.dram_tensor("wv8", (128, KC, 2, DPC), F8, kind="ExternalInput").ap()
    wo = nc.dram_tensor("wo", (DPC, D), F16, kind="ExternalInput").ap()
    ones = nc.dram_tensor("ones", (128, HPC), F16, kind="ExternalInput").ap()
    ident = nc.dram_tensor("ident", (128, 128), F16, kind="ExternalInput").ap()
    out = nc.dram_tensor("out", (T, D), F16, kind="ExternalOutput").ap()
    from contextlib import ExitStack

    with TileContext(nc) as tc, ExitStack() as st:
        emit_mha(tc, xh, xl, wq8, wk8, wv8, wo, ones, ident, out, st)
    nc.compile()
    _PROGRAM = nc
    return nc


def _split8(a):
    hi = np.clip(a, -240.0, 240.0).astype(E4NP)
    lo = np.clip(a - hi.astype(np.float32), -240.0, 240.0).astype(E4NP)
    return hi, lo


def _pack_qk(w, both_terms=True):
    # w: [DPC, D] torch-layout slice -> packed [128, MC, KC, (2,) 128] with
    # m-chunks contiguous per partition (one >=512B-elem DMA per m-chunk)
    wt = w.T.astype(np.float32) * WSCALE          # [D, DPC]
    hi, lo = _split8(wt)
    hi = hi.reshape(KC, 128, MC, 128)
    if not both_terms:
        return np.ascontiguousarray(hi.transpose(1, 2, 0, 3))
    lo = lo.reshape(KC, 128, MC, 128)
    arr = np.stack([hi, lo], axis=0)              # [2, c, p, m, j]
    return np.ascontiguousarray(arr.transpose(2, 3, 1, 0, 4))


def _pack_v(w):
    wt = w.T.astype(np.float32) * WSCALE
    hi, lo = _split8(wt)
    arr = np.stack([hi.reshape(KC, 128, DPC), lo.reshape(KC, 128, DPC)], axis=0)
    return np.ascontiguousarray(arr.transpose(2, 1, 0, 3))  # [p, c, 2, d]


def make_in_maps(x, Wq, Wk, Wv, Wo):
    x = np.asarray(x, dtype=np.float32)
    ones = np.full((128, HPC), ONES_VAL, np.float16)
    ident = np.eye(128, dtype=np.float16)
    xs = [_split8(x[b].T) for b in range(B)]
    xs = [(np.ascontiguousarray(h), np.ascontiguousarray(l)) for h, l in xs]
    in_maps = []
    for core in range(NCORES):
        b, hh = core // 2, core % 2
        sl = slice(hh * DPC, (hh + 1) * DPC)
        in_maps.append(
            {
                "xh": xs[b][0],
                "xl": xs[b][1],
                "wq8": _pack_qk(np.asarray(Wq)[sl]),
                "wk8": _pack_qk(np.asarray(Wk)[sl], both_terms=False),
                "wv8": _pack_v(np.asarray(Wv)[sl]),
                "wo": np.ascontiguousarray(np.asarray(Wo)[:, sl].T.astype(np.float16)),
                "ones": ones,
                "ident": ident,
            }
        )
    return in_maps


def kernel(x, Wq, Wk, Wv, Wo, bo):
    nc = build_program()
    in_maps = make_in_maps(x, Wq, Wk, Wv, Wo)
    res = run_bass_kernel_spmd(nc, in_maps, core_ids=list(range(NCORES)))
    bo = np.asarray(bo, dtype=np.float32)
    out = np.empty((B, T, D), dtype=np.float32)
    for b in range(B):
        out[b] = (res.results[2 * b]["out"].astype(np.float32)
                  + res.results[2 * b + 1]["out"].astype(np.float32) + bo)
    return out


# revision 22
# speedup vs baseline: 1.0134x; 1.0039x over previous
"""MultiHeadAttention Trainium2 kernel (8 NeuronCores, SPMD).

Reference computation (B=4, T=1024, D=768, H=12, Dh=64):
    q = x @ Wq.T ; k = x @ Wk.T ; v = x @ Wv.T       (per-head reshape)
    attn = softmax((q @ k.T) / 8)
    out = (attn @ v) @ Wo.T + bo
Sharding: 8 cores = 4 batches x 2 head-halves (6 heads each); host sums
the two partials per batch and adds the bias.

fp8 DoubleRow everywhere on the PE (0.5 cycles/output-column):
  - q: (hi, lo) compensation pair from a 2-term DR projection.
  - k: 1-term (w_hi only) DR projection, requantized to a duplicated
    single-fp8 pair (k feeds the scores matmul as fp8 anyway).
  - v: 3-term compensated projection (accuracy matters directly).
  - scores: DR with moving (q_hi, q_lo) pair vs stationary dup-k pair,
    fp32 PSUM tiles [128, 1024] (TRN2 matmul must write fp32 psum).
  - ctx + out_proj in fp16; flipped ctx layout [q_tile(128), 65] with a
    45.0 ones-column producing the softmax denominator as psum col 64.

ACT is the bottleneck engine: 48 exps x ~1.04us ~= 50us of exp stream.
The schedule keeps ACT saturated: packed host-side weight layouts make
every input DMA >= 512B/descriptor (fast head, first exp ~4.5us), the
48 scores units are interleaved with dependency-ordered PE filler work
(projections, flipped-ctx units, transposes, qt0 out tiles), and the
phase order A=(hp0,qt0) B=(1,0) C=(0,1) D=(2,0) E=(1,1) F=(2,1)
staggers the qt0/qt1 context phases so only pair-2-qt1's ctx units,
per-qs transposes, and the four qt1 out tiles remain after the final
exp. Within a phase the exps run hi-major so head hi=0's ctx units
start two exps early. Head k-psum copies ride ACT (idle pre-exp);
steady-state k plane-1 dups ride gpsimd (idle) to unload DVE.
"""

import numpy as np
import ml_dtypes

import concourse.mybir as mybir
from concourse import bacc
from concourse.tile import TileContext
from concourse.bass_utils import run_bass_kernel_spmd

FP = mybir.dt.float32
F16 = mybir.dt.float16
F8 = mybir.dt.float8e4
AF = mybir.ActivationFunctionType
DR = mybir.MatmulPerfMode.DoubleRow

E4NP = ml_dtypes.float8_e4m3

B, T, D = 4, 1024, 768
H, DH = 12, 64
NCORES = 8
HPC = 6           # heads per core
DPC = HPC * DH    # 384 head-dims per core
KC = D // 128     # 6 contraction chunks for d_in
CP = KC // 2      # 3 chunk-pairs for DoubleRow
MC = DPC // 128   # 3 chunks of per-core head dims (= head PAIRS)
NT = T // 512     # 2 free-dim tiles of tokens
TT = T // 128     # 8 partition tiles of tokens

WSCALE = 45.0                       # host scale on Wq/Wk/Wv
EXP_SCALE = 1.0 / (WSCALE * WSCALE * 8.0)  # S_psum = 45q . 45k = 16200*(qk/8)
ONES_VAL = WSCALE                   # denominator column matches the 45*v scale


def emit_mha(tc, xh, xl, wq8, wk8, wv8, wo, ones, ident, out, ctx):
    nc = tc.nc

    singles = ctx.enter_context(tc.tile_pool(name="singles", bufs=1))
    proj_psum = ctx.enter_context(tc.tile_pool(name="proj_psum", bufs=2, space="PSUM"))
    scores_psum = ctx.enter_context(
        tc.tile_pool(name="scores_psum", bufs=2, space="PSUM")
    )
    # ctx psum tiles share the proj pool (scores tiles need 3 banks x 2)
    ctx_psum = proj_psum
    expS_pool = ctx.enter_context(tc.tile_pool(name="expS", bufs=20))
    rcp_pool = ctx.enter_context(tc.tile_pool(name="rcp", bufs=10))
    ctxN_pool = ctx.enter_context(tc.tile_pool(name="ctxN", bufs=18))
    out_pool = ctx.enter_context(tc.tile_pool(name="outsb", bufs=6))

    # ---------------- staged input DMAs ----------------
    # All host tensors are pre-packed so every transfer moves >=512B per
    # descriptor (the cost model charges 2x below 512B). Weight m-chunks
    # are contiguous so the head-critical slices arrive in one descriptor
    # sweep each.
    xh_sb = singles.tile([128, KC, T], F8, name="xh_sb", tag="xh_sb")
    xl_sb = singles.tile([128, KC, T], F8, name="xl_sb", tag="xl_sb")
    wq_sb = singles.tile([128, MC, KC, 2, 128], F8, name="wq_sb", tag="wq_sb")
    wk_sb = singles.tile([128, MC, KC, 128], F8, name="wk_sb", tag="wk_sb")
    wv_sb = singles.tile([128, KC, 2, DPC], F8, name="wv_sb", tag="wv_sb")
    wo_sb = singles.tile([128, MC, D], F16, name="wo_sb", tag="wo_sb")
    ones_sb = singles.tile([128, HPC], F16, name="ones_sb", tag="ones_sb")
    ident_sb = singles.tile([128, 128], F16, name="ident_sb", tag="ident_sb")

    # critical chain for the first scores unit: xh n0, wq m0, wk m0.
    # Alternate issue sequencers so the queue is fed without stalling
    # ACT's sequencer (which must stay clear to decode the first exps).
    nc.sync.dma_start(out=xh_sb[:, :, 0:512], in_=xh.rearrange("(c p) t -> p c t", p=128)[:, :, 0:512])
    nc.scalar.dma_start(out=wq_sb[:, 0], in_=wq8[:, 0])
    nc.sync.dma_start(out=wk_sb[:, 0], in_=wk8[:, 0])
    nc.scalar.dma_start(out=xh_sb[:, :, 512:1024], in_=xh.rearrange("(c p) t -> p c t", p=128)[:, :, 512:1024])
    nc.gpsimd.dma_start(out=ones_sb, in_=ones)
    # non-critical: all on the SP sequencer
    nc.sync.dma_start(out=wq_sb[:, 1], in_=wq8[:, 1])
    nc.sync.dma_start(out=wk_sb[:, 1], in_=wk8[:, 1])
    nc.sync.dma_start(out=wq_sb[:, 2], in_=wq8[:, 2])
    nc.sync.dma_start(out=wk_sb[:, 2], in_=wk8[:, 2])
    nc.sync.dma_start(out=xl_sb[:, :, 0:512], in_=xl.rearrange("(c p) t -> p c t", p=128)[:, :, 0:512])
    nc.sync.dma_start(out=xl_sb[:, :, 512:1024], in_=xl.rearrange("(c p) t -> p c t", p=128)[:, :, 512:1024])
    nc.sync.dma_start(out=wv_sb, in_=wv8)
    nc.sync.dma_start(out=wo_sb, in_=wo.rearrange("(c p) d -> p c d", p=128))
    nc.sync.dma_start(out=ident_sb, in_=ident)

    # warm-up: dummy matmul chain keeps PE busy from ~1.2us until the
    # first inputs land so the cost model's 3us p-state ramp elapses on
    # throwaway work (the ramp clock resets whenever PE goes idle).
    wu_sb = singles.tile([128, 256], F16, name="wu_sb", tag="wu_sb")
    nc.vector.memset(wu_sb, 0.0)
    for _ in range(4):
        ps_wu = proj_psum.tile([128, 512], FP, name="ps_wu", tag="proj")
        nc.tensor.matmul(ps_wu[:, 0:256], lhsT=wu_sb[:, 0:128],
                         rhs=wu_sb[:, 0:256], start=True, stop=True)
        nc.tensor.matmul(ps_wu[:, 256:512], lhsT=wu_sb[:, 0:128],
                         rhs=wu_sb[:, 0:256], start=True, stop=True)

    q8_sb = singles.tile([128, MC, 2, T], F8, name="q8_sb", tag="q8_sb")
    k8_sb = singles.tile([128, MC, 2, T], F8, name="k8_sb", tag="k8_sb")
    ctxT_sb = singles.tile([128, MC, T], F16, name="ctxT_sb", tag="ctxT_sb")

    # v tiles [t_tile, 6 heads x (64 v cols + ones col)]: the 45.0 column
    # makes each head's ctx matmul also produce its softmax denominator.
    v_sb = [singles.tile([128, HPC, DH + 1], F16, name=f"v_sb{i}", tag=f"v_sb{i}")
            for i in range(TT)]

    def ones_fanout():
        for vt in v_sb:
            nc.gpsimd.tensor_copy(vt[:, :, DH : DH + 1], ones_sb)

    def qk_proj(m, n, dsts=("k", "q"), k_on_act=False, pool=None, fine=False):
        # q'/k' chunk m, token block n: psum[dout(128), t(512)]
        pool = pool or proj_psum
        sl = slice(n * 512, (n + 1) * 512)
        for d in dsts:
            ps = pool.tile([128, 512], FP, name="ps_qk", tag="proj")
            if d == "q":
                first = True
                for t in range(2):          # x_hi.w_hi + x_hi.w_lo
                    for cp in range(CP):
                        nc.tensor.matmul(
                            ps,
                            lhsT=wq_sb[:, m, 2 * cp : 2 * cp + 2, t, :],
                            rhs=xh_sb[:, 2 * cp : 2 * cp + 2, sl],
                            start=first,
                            stop=(t == 1 and cp == CP - 1),
                            perf_mode=DR,
                        )
                        first = False
                if fine:
                    # half-width copy+sub pairs so the split first scores
                    # unit can start on q columns 0:256 early
                    for h2 in range(2):
                        hsl = slice(n * 512 + h2 * 256, n * 512 + (h2 + 1) * 256)
                        psl = slice(h2 * 256, (h2 + 1) * 256)
                        nc.vector.tensor_copy(q8_sb[:, m, 0, hsl], ps[:, psl])
                        nc.vector.tensor_sub(q8_sb[:, m, 1, hsl], ps[:, psl],
                                             q8_sb[:, m, 0, hsl])
                else:
                    nc.vector.tensor_copy(q8_sb[:, m, 0, sl], ps)
                    nc.vector.tensor_sub(q8_sb[:, m, 1, sl], ps, q8_sb[:, m, 0, sl])
            else:
                # k: 1-term (w_hi only) -- k is requantized to single fp8
                # for the scores matmul, so w_lo precision is wasted there
                first = True
                for cp in range(CP):
                    nc.tensor.matmul(
                        ps,
                        lhsT=wk_sb[:, m, 2 * cp : 2 * cp + 2, :],
                        rhs=xh_sb[:, 2 * cp : 2 * cp + 2, sl],
                        start=first,
                        stop=(cp == CP - 1),
                        perf_mode=DR,
                    )
                    first = False
                if k_on_act:
                    # head region: ACT is idle pre-exp; both plane copies
                    # run parallel to the q copies on DVE. Split at token
                    # 384 so k-tiles 0-2 (the first scores unit) go first.
                    for ksl, psl in ((slice(0, 384), slice(0, 384)),
                                     (slice(384, 512), slice(384, 512))):
                        nc.scalar.copy(k8_sb[:, m, 0, ksl], ps[:, psl])
                        nc.scalar.copy(k8_sb[:, m, 1, ksl], ps[:, psl])
                else:
                    nc.vector.tensor_copy(k8_sb[:, m, 0, sl], ps)
                    # plane-1 dup from SBUF on gpsimd (idle; DVE relief)
                    nc.gpsimd.tensor_copy(k8_sb[:, m, 1, sl], k8_sb[:, m, 0, sl])

    def v_proj(mts):
        # v': psum[t_tile(128), dh(384)] = 45 * sum_c x[c].T wv[c]
        for mt in mts:
            ps = proj_psum.tile([128, DPC], FP, name="ps_v", tag="proj")
            first = True
            for xt, t in ((xh_sb, 0), (xh_sb, 1), (xl_sb, 0)):
                for cp in range(CP):
                    nc.tensor.matmul(
                        ps,
                        lhsT=xt[:, 2 * cp : 2 * cp + 2, mt * 128 : (mt + 1) * 128],
                        rhs=wv_sb[:, 2 * cp : 2 * cp + 2, t, :],
                        start=first,
                        stop=(xt is xl_sb and cp == CP - 1),
                        perf_mode=DR,
                    )
                    first = False
            nc.vector.tensor_copy(v_sb[mt][:, :, 0:DH], ps)

    # exp tiles per (hp, qt, hi): k-tile groups (0-2, 3-5, 6-7); the wide
    # tiles amortize the ~185ns/inst PSUM/SBUF access overhead on ACT over
    # 1536 columns instead of 1024
    KGRP = ((0, 1, 2), (3, 4, 5), (6, 7))
    exps = {}

    def scores_unit_split(hp, qt, hi, g):
        # head variant: same psum tile, but matmuls and exp run per
        # 256-col q-half so the exp stream starts on half the q copies
        po = 64 * hi
        js = KGRP[g]
        ps = scores_psum.tile([128, len(js), 512], FP, name="ps_s", tag="scores")
        halves = []
        for h2 in range(2):
            for r, j in enumerate(js):
                nc.tensor.matmul(
                    ps[:, r, h2 * 256 : (h2 + 1) * 256],
                    lhsT=k8_sb[po : po + 64, hp, :, j * 128 : (j + 1) * 128],
                    rhs=q8_sb[po : po + 64, hp, :,
                              qt * 512 + h2 * 256 : qt * 512 + (h2 + 1) * 256],
                    start=True,
                    stop=True,
                    perf_mode=DR,
                )
            ex = expS_pool.tile([128, len(js), 256], F16, name="exh", tag="expS")
            nc.scalar.activation(ex, ps[:, :, h2 * 256 : (h2 + 1) * 256],
                                 AF.Exp, scale=EXP_SCALE)
            halves.append(ex)
        exps.setdefault((hp, qt, hi), [None] * 3)[g] = tuple(halves)

    def scores_unit(hp, qt, hi, g):
        po = 64 * hi
        js = KGRP[g]
        ps = scores_psum.tile([128, 512 * len(js)], FP, name="ps_s", tag="scores")
        for r, j in enumerate(js):
            nc.tensor.matmul(
                ps[:, r * 512 : (r + 1) * 512],
                lhsT=k8_sb[po : po + 64, hp, :, j * 128 : (j + 1) * 128],
                rhs=q8_sb[po : po + 64, hp, :, qt * 512 : (qt + 1) * 512],
                start=True,
                stop=True,
                perf_mode=DR,
            )
        ex = expS_pool.tile([128, 512 * len(js)], F16, name="ex", tag="expS")
        nc.scalar.activation(ex, ps, AF.Exp, scale=EXP_SCALE)
        exps.setdefault((hp, qt, hi), [None] * 3)[g] = ex

    cn_store = {}

    def ctx_q(hp, qt, hi, qs):
        # flipped ctx: out[q_tile(128), 65] = sum_kt expST[kt, q].T @ [45v|45]
        ex = exps[(hp, qt, hi)]
        h = 2 * hp + hi
        pc = ctx_psum.tile([128, 65], FP, name="pcq", tag="proj")
        c0 = qs * 128
        for j in range(TT):
            g, r = (j // 3, j % 3) if j < 6 else (2, j - 6)
            if isinstance(ex[g], tuple):
                lhsT = ex[g][qs // 2][:, r, (qs % 2) * 128 : (qs % 2) * 128 + 128]
            else:
                lhsT = ex[g][:, r * 512 + c0 : r * 512 + c0 + 128]
            nc.tensor.matmul(
                pc,
                lhsT=lhsT,
                rhs=v_sb[j][:, h, :],
                start=(j == 0),
                stop=(j == TT - 1),
            )
        rcp = rcp_pool.tile([128, 1], FP, name="rcp", tag="rcp")
        cn = ctxN_pool.tile([128, DH], F16, name="ctxN", tag="ctxN")
        nc.vector.reciprocal(rcp, pc[:, DH : DH + 1])
        nc.vector.tensor_scalar_mul(cn, pc[:, 0:DH], rcp)
        cn_store.setdefault((hp, qt), [[None] * 4 for _ in range(2)])[hi][qs] = cn

    def ctx_t(hp, qt):
        # batched transpose of the pair's eight [q(128), dh(64)] tiles back
        # into ctxT layout via one fp16 psum + a single 2x-mode DVE copy
        cns = cn_store[(hp, qt)]
        pt = proj_psum.tile([128, 512], F16, name="pt", tag="proj")
        for hi in range(2):
            po = 64 * hi
            for qs in range(4):
                nc.tensor.transpose(
                    pt[po : po + 64, qs * 128 : (qs + 1) * 128],
                    cns[hi][qs],
                    ident_sb,
                )
        nc.vector.tensor_copy(ctxT_sb[:, hp, qt * 512 : (qt + 1) * 512], pt)

    def ctx_t_qs(hp, qt, qs):
        # tail variant: per-qs transpose so out tile mt=4qt+qs unblocks
        # right after its own q-slice, not after the whole pair
        cns = cn_store[(hp, qt)]
        pt = proj_psum.tile([128, 128], F16, name="ptq", tag="proj")
        for hi in range(2):
            po = 64 * hi
            nc.tensor.transpose(pt[po : po + 64, :], cns[hi][qs], ident_sb)
        nc.vector.tensor_copy(
            ctxT_sb[:, hp, qt * 512 + qs * 128 : qt * 512 + (qs + 1) * 128], pt)

    # paired output staging: tiles (2i, 2i+1) share one [128, 2, 768] SBUF
    # buffer and leave in ONE dma (each dma_start costs ~630ns of exclusive
    # HWDGE issue time, so halving the count shortens the tail directly)
    osb_pairs = [singles.tile([128, 2, D], F16, name=f"osb{i}", tag=f"osb{i}")
                 for i in range(4)]

    def out_proj(mt, tail=False, n2s=(0, 1)):
        # out[t_tile(128), dout(768)] = sum_c ctxT16[c].T @ wo16[c] in two
        # 384-col halves. Tail tiles borrow the (dead by then) scores psum
        # pool for the second half and put that copy on ACT (idle post-exp).
        osb = osb_pairs[mt // 2]
        for n2 in n2s:
            pool = scores_psum if (tail and n2 == 1) else proj_psum
            ps = pool.tile([128, 384], FP, name="ps_o",
                           tag="proj" if pool is proj_psum else "scores")
            for c in range(MC):
                nc.tensor.matmul(
                    ps,
                    lhsT=ctxT_sb[:, c, mt * 128 : (mt + 1) * 128],
                    rhs=wo_sb[:, c, n2 * 384 : (n2 + 1) * 384],
                    start=(c == 0),
                    stop=(c == MC - 1),
                )
            csl = slice(n2 * 384, (n2 + 1) * 384)
            if tail and n2 == 0:
                # ACT is idle once the exp stream drains; splitting the two
                # halves across ACT/DVE halves the copy latency per tile
                nc.scalar.copy(osb[:, mt % 2, csl], ps)
            else:
                nc.vector.tensor_copy(osb[:, mt % 2, csl], ps)
        if 1 not in n2s:
            return
        orr = out.rearrange("(b p) d -> p b d", p=128)
        if mt >= 6:
            # last two tiles leave individually so mt6's transfer overlaps
            # mt7's compute and the final DMA is a short one
            nc.sync.dma_start(out=orr[:, mt : mt + 1, :],
                              in_=osb[:, mt % 2 : mt % 2 + 1, :])
        elif mt % 2 == 1:
            # pair complete: one DMA for rows (mt-1)*128 .. (mt+1)*128
            nc.sync.dma_start(out=orr[:, mt - 1 : mt + 1, :], in_=osb)

    # ---------------- schedule ----------------
    # Phase order staggers qt so ctx/transpose/out work spreads out:
    #   A=(0,0) B=(1,0) C=(0,1) D=(2,0) E=(1,1) F=(2,1)
    # Units within a phase run hi-major (hi0 g0..3, hi1 g0..3) so the
    # hi0 ctx units become available four exps before the phase ends.
    def phase_units(hp, qt):
        return [(hp, qt, hi, g) for hi in range(2) for g in range(3)]

    # head: critical qk m0/n0 (k copies on ACT -- idle pre-exp), phase-A
    # units interleaved with the remaining projection groups
    # head: unit (hi, g=0) needs only k/q n0; the n1 groups have two
    # exps (~3us) of slack before unit g=1 (k-tiles 3-5 span both halves)
    qk_proj(0, 0, k_on_act=True, fine=True)
    scores_unit_split(0, 0, 0, 0)
    scores_unit(0, 0, 1, 0)
    qk_proj(0, 1, dsts=("k",))
    qk_proj(0, 1, dsts=("q",))
    scores_unit(0, 0, 0, 1)
    qk_proj(1, 0, dsts=("k",))
    scores_unit(0, 0, 1, 1)
    qk_proj(1, 0, dsts=("q",))
    scores_unit(0, 0, 0, 2)
    ones_fanout()
    scores_unit(0, 0, 1, 2)

    stream = (phase_units(1, 0) + phase_units(0, 1) + phase_units(2, 0)
              + phase_units(1, 1) + phase_units(2, 1))

    # filler groups in dependency order, consumed one per scores unit
    # (each scores unit is ~0.21us PE vs a ~1.04us exp, so PE has ~0.8us
    # of filler headroom per unit)
    fillers = [
        # during B: remaining projections, v tiles
        [lambda: qk_proj(1, 1)],
        [lambda: qk_proj(2, 0)],
        [lambda: qk_proj(2, 1)],
        [lambda: v_proj([0])],
        [lambda: v_proj([1])],
        [lambda: v_proj([2])],
        # during C: rest of v, ctxA
        [lambda: v_proj([3])],
        [lambda: v_proj([4])],
        [lambda: v_proj([5])],
        [lambda: v_proj([6])],
        [lambda: v_proj([7])],
        [lambda qs=qs: ctx_q(0, 0, 0, qs) for qs in range(4)],
        # during D: ctxA hi1, ctxB, ctxC
        [lambda qs=qs: ctx_q(0, 0, 1, qs) for qs in range(4)] + [lambda: ctx_t(0, 0)],
        [lambda qs=qs: ctx_q(1, 0, 0, qs) for qs in range(4)],
        [lambda qs=qs: ctx_q(1, 0, 1, qs) for qs in range(4)] + [lambda: ctx_t(1, 0)],
        [lambda qs=qs: ctx_q(0, 1, 0, qs) for qs in range(4)],
        [lambda qs=qs: ctx_q(0, 1, 1, qs) for qs in range(4)] + [lambda: ctx_t(0, 1)],
        [],
        # during E: ctxD, out qt0 (half-tile groups: one psum+copy per
        # slot keeps the shared proj/ctx pool from head-of-line blocking
        # the scores units)
        [lambda qs=qs: ctx_q(2, 0, 0, qs) for qs in range(4)],
        [lambda qs=qs: ctx_q(2, 0, 1, qs) for qs in range(4)] + [lambda: ctx_t(2, 0)],
        [lambda: out_proj(0, n2s=(0,))],
        [lambda: out_proj(0, n2s=(1,))],
        [lambda: out_proj(1, n2s=(0,))],
        [lambda: out_proj(1, n2s=(1,))],
        # during F: ctxE, out qt0 tail, ctxF hi0
        [lambda qs=qs: ctx_q(1, 1, 0, qs) for qs in range(4)],
        [lambda qs=qs: ctx_q(1, 1, 1, qs) for qs in range(4)] + [lambda: ctx_t(1, 1)],
        [lambda: out_proj(2, n2s=(0,)), lambda: out_proj(2, n2s=(1,))],
        [lambda qs=qs: ctx_q(2, 1, 0, qs) for qs in range(4)],
        [lambda: out_proj(3, n2s=(0,))],
        [lambda: out_proj(3, n2s=(1,))],
    ]

    fi = iter(fillers)
    for u in stream:
        scores_unit(*u)
        grp = next(fi, None)
        if grp is not None:
            for f in grp:
                f()
    for grp in fi:
        for f in grp:
            f()

    # tail: after F's last exp only pair-2-qt1's hi1 ctx units, per-qs
    # transposes, and the qt1 out tiles remain. Emission is software-
    # pipelined so PE never waits on a DVE round-trip: while out tile qs
    # waits on its ctxT copy, PE runs ctx/transpose work for qs+1.
    ctx_q(2, 1, 1, 0)
    ctx_q(2, 1, 1, 1)
    ctx_t_qs(2, 1, 0)
    ctx_q(2, 1, 1, 2)
    ctx_t_qs(2, 1, 1)
    out_proj(4, tail=True)
    ctx_q(2, 1, 1, 3)
    ctx_t_qs(2, 1, 2)
    out_proj(5, tail=True)
    ctx_t_qs(2, 1, 3)
    out_proj(6, tail=True)
    out_proj(7, tail=True)


_PROGRAM = None


def build_program():
    global _PROGRAM
    if _PROGRAM is not None:
        return _PROGRAM
    nc = bacc.Bacc("TRN2", target_bir_lowering=False, debug=False, num_devices=NCORES)
    xh = nc.dram_tensor("xh", (D, T), F8, kind="ExternalInput").ap()
    xl = nc.dram_tensor("xl", (D, T), F8, kind="ExternalInput").ap()
    wq8 = nc.dram_tensor("wq8", (128, MC, KC, 2, 128), F8, kind="ExternalInput").ap()
    wk8 = nc.dram_tensor("wk8", (128, MC, KC, 128), F8, kind="ExternalInput").ap()
    wv8 = nc.dram_tensor("wv8", (128, KC, 2, DPC), F8, kind="ExternalInput").ap()
    wo = nc.dram_tensor("wo", (DPC, D), F16, kind="ExternalInput").ap()
    ones = nc.dram_tensor("ones", (128, HPC), F16, kind="ExternalInput").ap()
    ident = nc.dram_tensor("ident", (128, 128), F16, kind="ExternalInput").ap()
    out = nc.dram_tensor("out", (T, D), F16, kind="ExternalOutput").ap()
    from contextlib import ExitStack

    with TileContext(nc) as tc, ExitStack() as st:
        emit_mha(tc, xh, xl, wq8, wk8, wv8, wo, ones, ident, out, st)
    nc.compile()
    _PROGRAM = nc
    return nc


def _split8(a):
    hi = np.clip(a, -240.0, 240.0).astype(E4NP)
    lo = np.clip(a - hi.astype(np.float32), -240.0, 240.0).astype(E4NP)
    return hi, lo


def _pack_qk(w, both_terms=True):
    # w: [DPC, D] torch-layout slice -> packed [128, MC, KC, (2,) 128] with
    # m-chunks contiguous per partition (one >=512B-elem DMA per m-chunk)
    wt = w.T.astype(np.float32) * WSCALE          # [D, DPC]
    hi, lo = _split8(wt)
    hi = hi.reshape(KC, 128, MC, 128)
    if not both_terms:
        return np.ascontiguousarray(hi.transpose(1, 2, 0, 3))
    lo = lo.reshape(KC, 128, MC, 128)
    arr = np.stack([hi, lo], axis=0)              # [2, c, p, m, j]
    return np.ascontiguousarray(arr.transpose(2, 3, 1, 0, 4))


def _pack_v(w):
    wt = w.T.astype(np.float32) * WSCALE
    hi, lo = _split8(wt)
    arr = np.stack([hi.reshape(KC, 128, DPC), lo.reshape(KC, 128, DPC)], axis=0)
    return np.ascontiguousarray(arr.transpose(2, 1, 0, 3))  # [p, c, 2, d]


def make_in_maps(x, Wq, Wk, Wv, Wo):
    x = np.asarray(x, dtype=np.float32)
    ones = np.full((128, HPC), ONES_VAL, np.float16)
    ident = np.eye(128, dtype=np.float16)
    xs = [_split8(x[b].T) for b in range(B)]
    xs = [(np.ascontiguousarray(h), np.ascontiguousarray(l)) for h, l in xs]
    in_maps = []
    for core in range(NCORES):
        b, hh = core // 2, core % 2
        sl = slice(hh * DPC, (hh + 1) * DPC)
        in_maps.append(
            {
                "xh": xs[b][0],
                "xl": xs[b][1],
                "wq8": _pack_qk(np.asarray(Wq)[sl]),
                "wk8": _pack_qk(np.asarray(Wk)[sl], both_terms=False),
                "wv8": _pack_v(np.asarray(Wv)[sl]),
                "wo": np.ascontiguousarray(np.asarray(Wo)[:, sl].T.astype(np.float16)),
                "ones": ones,
                "ident": ident,
            }
        )
    return in_maps


def kernel(x, Wq, Wk, Wv, Wo, bo):
    nc = build_program()
    in_maps = make_in_maps(x, Wq, Wk, Wv, Wo)
    res = run_bass_kernel_spmd(nc, in_maps, core_ids=list(range(NCORES)))
    bo = np.asarray(bo, dtype=np.float32)
    out = np.empty((B, T, D), dtype=np.float32)
    for b in range(B):
        out[b] = (res.results[2 * b]["out"].astype(np.float32)
                  + res.results[2 * b + 1]["out"].astype(np.float32) + bo)
    return out


# revision 26
# speedup vs baseline: 1.0165x; 1.0030x over previous
"""MultiHeadAttention Trainium2 kernel (8 NeuronCores, SPMD).

Reference computation (B=4, T=1024, D=768, H=12, Dh=64):
    q = x @ Wq.T ; k = x @ Wk.T ; v = x @ Wv.T       (per-head reshape)
    attn = softmax((q @ k.T) / 8)
    out = (attn @ v) @ Wo.T + bo
Sharding: 8 cores = 4 batches x 2 head-halves (6 heads each); host sums
the two partials per batch and adds the bias.

fp8 DoubleRow everywhere on the PE (0.5 cycles/output-column):
  - q: (hi, lo) compensation pair from a 2-term DR projection.
  - k: 1-term (w_hi only) DR projection, requantized to a duplicated
    single-fp8 pair (k feeds the scores matmul as fp8 anyway).
  - v: 3-term compensated projection (accuracy matters directly).
  - scores: DR with moving (q_hi, q_lo) pair vs stationary dup-k pair,
    fp32 PSUM tiles [128, 1024] (TRN2 matmul must write fp32 psum).
  - ctx + out_proj in fp16; flipped ctx layout [q_tile(128), 65] with a
    45.0 ones-column producing the softmax denominator as psum col 64.

ACT is the bottleneck engine: 48 exps x ~1.04us ~= 50us of exp stream.
The schedule keeps ACT saturated: packed host-side weight layouts make
every input DMA >= 512B/descriptor (fast head, first exp ~4.5us), the
48 scores units are interleaved with dependency-ordered PE filler work
(projections, flipped-ctx units, transposes, qt0 out tiles), and the
phase order A=(hp0,qt0) B=(1,0) C=(0,1) D=(2,0) E=(1,1) F=(2,1)
staggers the qt0/qt1 context phases so only pair-2-qt1's ctx units,
per-qs transposes, and the four qt1 out tiles remain after the final
exp. Within a phase the exps run hi-major so head hi=0's ctx units
start two exps early. Head k-psum copies ride ACT (idle pre-exp);
steady-state k plane-1 dups ride gpsimd (idle) to unload DVE.
"""

import numpy as np
import ml_dtypes

import concourse.mybir as mybir
from concourse import bacc
from concourse.tile import TileContext
from concourse.bass_utils import run_bass_kernel_spmd

FP = mybir.dt.float32
F16 = mybir.dt.float16
F8 = mybir.dt.float8e4
AF = mybir.ActivationFunctionType
DR = mybir.MatmulPerfMode.DoubleRow

E4NP = ml_dtypes.float8_e4m3

B, T, D = 4, 1024, 768
H, DH = 12, 64
NCORES = 8
HPC = 6           # heads per core
DPC = HPC * DH    # 384 head-dims per core
KC = D // 128     # 6 contraction chunks for d_in
CP = KC // 2      # 3 chunk-pairs for DoubleRow
MC = DPC // 128   # 3 chunks of per-core head dims (= head PAIRS)
NT = T // 512     # 2 free-dim tiles of tokens
TT = T // 128     # 8 partition tiles of tokens

WSCALE = 45.0                       # host scale on Wq/Wk/Wv
EXP_SCALE = 1.0 / (WSCALE * WSCALE * 8.0)  # S_psum = 45q . 45k = 16200*(qk/8)
ONES_VAL = WSCALE                   # denominator column matches the 45*v scale


def emit_mha(tc, xh, xl, wq8, wk8, wv8, wo, ones, ident, out, ctx):
    nc = tc.nc

    singles = ctx.enter_context(tc.tile_pool(name="singles", bufs=1))
    proj_psum = ctx.enter_context(tc.tile_pool(name="proj_psum", bufs=2, space="PSUM"))
    scores_psum = ctx.enter_context(
        tc.tile_pool(name="scores_psum", bufs=2, space="PSUM")
    )
    # ctx psum tiles share the proj pool (scores tiles need 3 banks x 2)
    ctx_psum = proj_psum
    expS_pool = ctx.enter_context(tc.tile_pool(name="expS", bufs=20))
    rcp_pool = ctx.enter_context(tc.tile_pool(name="rcp", bufs=10))
    ctxN_pool = ctx.enter_context(tc.tile_pool(name="ctxN", bufs=18))
    out_pool = ctx.enter_context(tc.tile_pool(name="outsb", bufs=6))

    # ---------------- staged input DMAs ----------------
    # All host tensors are pre-packed so every transfer moves >=512B per
    # descriptor (the cost model charges 2x below 512B). Weight m-chunks
    # are contiguous so the head-critical slices arrive in one descriptor
    # sweep each.
    xh_sb = singles.tile([128, KC, T], F8, name="xh_sb", tag="xh_sb")
    xl_sb = singles.tile([128, KC, T], F8, name="xl_sb", tag="xl_sb")
    wq_sb = singles.tile([128, MC, KC, 2, 128], F8, name="wq_sb", tag="wq_sb")
    wk_sb = singles.tile([128, MC, KC, 128], F8, name="wk_sb", tag="wk_sb")
    wv_sb = singles.tile([128, KC, 2, DPC], F8, name="wv_sb", tag="wv_sb")
    wo_sb = singles.tile([128, MC, D], F16, name="wo_sb", tag="wo_sb")
    ones_sb = singles.tile([128, HPC], F16, name="ones_sb", tag="ones_sb")
    ident_sb = singles.tile([128, 128], F16, name="ident_sb", tag="ident_sb")

    # critical chain for the first scores unit: xh n0, wq m0, wk m0.
    # Alternate issue sequencers so the queue is fed without stalling
    # ACT's sequencer (which must stay clear to decode the first exps).
    nc.sync.dma_start(out=xh_sb[:, :, 0:512], in_=xh.rearrange("(c p) t -> p c t", p=128)[:, :, 0:512])
    nc.scalar.dma_start(out=wq_sb[:, 0], in_=wq8[:, 0])
    nc.sync.dma_start(out=wk_sb[:, 0], in_=wk8[:, 0])
    nc.scalar.dma_start(out=xh_sb[:, :, 512:1024], in_=xh.rearrange("(c p) t -> p c t", p=128)[:, :, 512:1024])
    nc.gpsimd.dma_start(out=ones_sb, in_=ones)
    # non-critical: all on the SP sequencer
    nc.sync.dma_start(out=wq_sb[:, 1], in_=wq8[:, 1])
    nc.sync.dma_start(out=wk_sb[:, 1], in_=wk8[:, 1])
    nc.sync.dma_start(out=wq_sb[:, 2], in_=wq8[:, 2])
    nc.sync.dma_start(out=wk_sb[:, 2], in_=wk8[:, 2])
    nc.sync.dma_start(out=xl_sb[:, :, 0:512], in_=xl.rearrange("(c p) t -> p c t", p=128)[:, :, 0:512])
    nc.sync.dma_start(out=xl_sb[:, :, 512:1024], in_=xl.rearrange("(c p) t -> p c t", p=128)[:, :, 512:1024])
    nc.sync.dma_start(out=wv_sb, in_=wv8)
    nc.sync.dma_start(out=wo_sb, in_=wo.rearrange("(c p) d -> p c d", p=128))
    nc.sync.dma_start(out=ident_sb, in_=ident)

    # warm-up: dummy matmul chain keeps PE busy from ~1.2us until the
    # first inputs land so the cost model's 3us p-state ramp elapses on
    # throwaway work (the ramp clock resets whenever PE goes idle).
    wu_sb = singles.tile([128, 256], F16, name="wu_sb", tag="wu_sb")
    nc.vector.memset(wu_sb, 0.0)
    for _ in range(4):
        ps_wu = proj_psum.tile([128, 512], FP, name="ps_wu", tag="proj")
        nc.tensor.matmul(ps_wu[:, 0:256], lhsT=wu_sb[:, 0:128],
                         rhs=wu_sb[:, 0:256], start=True, stop=True)
        nc.tensor.matmul(ps_wu[:, 256:512], lhsT=wu_sb[:, 0:128],
                         rhs=wu_sb[:, 0:256], start=True, stop=True)

    q8_sb = singles.tile([128, MC, 2, T], F8, name="q8_sb", tag="q8_sb")
    k8_sb = singles.tile([128, MC, 2, T], F8, name="k8_sb", tag="k8_sb")
    ctxT_sb = singles.tile([128, MC, T], F16, name="ctxT_sb", tag="ctxT_sb")

    # v tiles [t_tile, 6 heads x (64 v cols + ones col)]: the 45.0 column
    # makes each head's ctx matmul also produce its softmax denominator.
    v_sb = [singles.tile([128, HPC, DH + 1], F16, name=f"v_sb{i}", tag=f"v_sb{i}")
            for i in range(TT)]

    def ones_fanout():
        for vt in v_sb:
            nc.gpsimd.tensor_copy(vt[:, :, DH : DH + 1], ones_sb)

    def qk_proj(m, n, dsts=("k", "q"), k_on_act=False, pool=None, fine=False):
        # q'/k' chunk m, token block n: psum[dout(128), t(512)]
        pool = pool or proj_psum
        sl = slice(n * 512, (n + 1) * 512)
        for d in dsts:
            ps = pool.tile([128, 512], FP, name="ps_qk", tag="proj")
            if d == "q":
                first = True
                for t in range(2):          # x_hi.w_hi + x_hi.w_lo
                    for cp in range(CP):
                        nc.tensor.matmul(
                            ps,
                            lhsT=wq_sb[:, m, 2 * cp : 2 * cp + 2, t, :],
                            rhs=xh_sb[:, 2 * cp : 2 * cp + 2, sl],
                            start=first,
                            stop=(t == 1 and cp == CP - 1),
                            perf_mode=DR,
                        )
                        first = False
                if fine:
                    # half-width copy+sub pairs so the split first scores
                    # unit can start on q columns 0:256 early
                    for h2 in range(2):
                        hsl = slice(n * 512 + h2 * 256, n * 512 + (h2 + 1) * 256)
                        psl = slice(h2 * 256, (h2 + 1) * 256)
                        nc.vector.tensor_copy(q8_sb[:, m, 0, hsl], ps[:, psl])
                        nc.vector.tensor_sub(q8_sb[:, m, 1, hsl], ps[:, psl],
                                             q8_sb[:, m, 0, hsl])
                else:
                    nc.vector.tensor_copy(q8_sb[:, m, 0, sl], ps)
                    nc.vector.tensor_sub(q8_sb[:, m, 1, sl], ps, q8_sb[:, m, 0, sl])
            else:
                # k: 1-term (w_hi only) -- k is requantized to single fp8
                # for the scores matmul, so w_lo precision is wasted there
                first = True
                for cp in range(CP):
                    nc.tensor.matmul(
                        ps,
                        lhsT=wk_sb[:, m, 2 * cp : 2 * cp + 2, :],
                        rhs=xh_sb[:, 2 * cp : 2 * cp + 2, sl],
                        start=first,
                        stop=(cp == CP - 1),
                        perf_mode=DR,
                    )
                    first = False
                if k_on_act:
                    # head region: ACT is idle pre-exp; both plane copies
                    # run parallel to the q copies on DVE. Split at token
                    # 384 so k-tiles 0-2 (the first scores unit) go first.
                    for ksl, psl in ((slice(0, 384), slice(0, 384)),
                                     (slice(384, 512), slice(384, 512))):
                        nc.scalar.copy(k8_sb[:, m, 0, ksl], ps[:, psl])
                        nc.scalar.copy(k8_sb[:, m, 1, ksl], ps[:, psl])
                else:
                    nc.vector.tensor_copy(k8_sb[:, m, 0, sl], ps)
                    # plane-1 dup from SBUF on gpsimd (idle; DVE relief)
                    nc.gpsimd.tensor_copy(k8_sb[:, m, 1, sl], k8_sb[:, m, 0, sl])

    def v_proj(mts):
        # v': psum[t_tile(128), dh(384)] = 45 * sum_c x[c].T wv[c]
        for mt in mts:
            ps = proj_psum.tile([128, DPC], FP, name="ps_v", tag="proj")
            first = True
            for xt, t in ((xh_sb, 0), (xh_sb, 1), (xl_sb, 0)):
                for cp in range(CP):
                    nc.tensor.matmul(
                        ps,
                        lhsT=xt[:, 2 * cp : 2 * cp + 2, mt * 128 : (mt + 1) * 128],
                        rhs=wv_sb[:, 2 * cp : 2 * cp + 2, t, :],
                        start=first,
                        stop=(xt is xl_sb and cp == CP - 1),
                        perf_mode=DR,
                    )
                    first = False
            nc.vector.tensor_copy(v_sb[mt][:, :, 0:DH], ps)

    # exp tiles per (hp, qt, hi): k-tile groups (0-2, 3-5, 6-7); the wide
    # tiles amortize the ~185ns/inst PSUM/SBUF access overhead on ACT over
    # 1536 columns instead of 1024
    KGRP = ((0, 1, 2), (3, 4, 5), (6, 7))
    exps = {}

    def scores_unit_split(hp, qt, hi, g):
        # split variant: matmuls and exp run per 256-col q-half so the
        # first half-exp can start before the second half's inputs are
        # ready (head: q arrives in halves; tail: overlaps the last exp)
        po = 64 * hi
        js = KGRP[g]
        ps = scores_psum.tile([128, len(js), 512], FP, name="ps_s", tag="scores")
        halves = []
        for h2 in range(2):
            for r, j in enumerate(js):
                nc.tensor.matmul(
                    ps[:, r, h2 * 256 : (h2 + 1) * 256],
                    lhsT=k8_sb[po : po + 64, hp, :, j * 128 : (j + 1) * 128],
                    rhs=q8_sb[po : po + 64, hp, :,
                              qt * 512 + h2 * 256 : qt * 512 + (h2 + 1) * 256],
                    start=True,
                    stop=True,
                    perf_mode=DR,
                )
            ex = expS_pool.tile([128, len(js), 256], F16, name="exh", tag="expS")
            nc.scalar.activation(ex, ps[:, :, h2 * 256 : (h2 + 1) * 256],
                                 AF.Exp, scale=EXP_SCALE)
            halves.append(ex)
        exps.setdefault((hp, qt, hi), [None] * 3)[g] = tuple(halves)

    def scores_unit(hp, qt, hi, g):
        po = 64 * hi
        js = KGRP[g]
        ps = scores_psum.tile([128, 512 * len(js)], FP, name="ps_s", tag="scores")
        for r, j in enumerate(js):
            nc.tensor.matmul(
                ps[:, r * 512 : (r + 1) * 512],
                lhsT=k8_sb[po : po + 64, hp, :, j * 128 : (j + 1) * 128],
                rhs=q8_sb[po : po + 64, hp, :, qt * 512 : (qt + 1) * 512],
                start=True,
                stop=True,
                perf_mode=DR,
            )
        ex = expS_pool.tile([128, 512 * len(js)], F16, name="ex", tag="expS")
        nc.scalar.activation(ex, ps, AF.Exp, scale=EXP_SCALE)
        exps.setdefault((hp, qt, hi), [None] * 3)[g] = ex

    cn_store = {}

    def ctx_q(hp, qt, hi, qs):
        # flipped ctx: out[q_tile(128), 65] = sum_kt expST[kt, q].T @ [45v|45]
        ex = exps[(hp, qt, hi)]
        h = 2 * hp + hi
        pc = ctx_psum.tile([128, 65], FP, name="pcq", tag="proj")
        c0 = qs * 128
        for j in range(TT):
            g, r = (j // 3, j % 3) if j < 6 else (2, j - 6)
            if isinstance(ex[g], tuple):
                lhsT = ex[g][qs // 2][:, r, (qs % 2) * 128 : (qs % 2) * 128 + 128]
            else:
                lhsT = ex[g][:, r * 512 + c0 : r * 512 + c0 + 128]
            nc.tensor.matmul(
                pc,
                lhsT=lhsT,
                rhs=v_sb[j][:, h, :],
                start=(j == 0),
                stop=(j == TT - 1),
            )
        rcp = rcp_pool.tile([128, 1], FP, name="rcp", tag="rcp")
        cn = ctxN_pool.tile([128, DH], F16, name="ctxN", tag="ctxN")
        nc.vector.reciprocal(rcp, pc[:, DH : DH + 1])
        nc.vector.tensor_scalar_mul(cn, pc[:, 0:DH], rcp)
        cn_store.setdefault((hp, qt), [[None] * 4 for _ in range(2)])[hi][qs] = cn

    def ctx_t(hp, qt):
        # batched transpose of the pair's eight [q(128), dh(64)] tiles back
        # into ctxT layout via one fp16 psum + a single 2x-mode DVE copy
        cns = cn_store[(hp, qt)]
        pt = proj_psum.tile([128, 512], F16, name="pt", tag="proj")
        for hi in range(2):
            po = 64 * hi
            for qs in range(4):
                nc.tensor.transpose(
                    pt[po : po + 64, qs * 128 : (qs + 1) * 128],
                    cns[hi][qs],
                    ident_sb,
                )
        nc.vector.tensor_copy(ctxT_sb[:, hp, qt * 512 : (qt + 1) * 512], pt)

    def ctx_t_qs(hp, qt, qs):
        # tail variant: per-qs transpose so out tile mt=4qt+qs unblocks
        # right after its own q-slice, not after the whole pair
        cns = cn_store[(hp, qt)]
        pt = proj_psum.tile([128, 128], F16, name="ptq", tag="proj")
        for hi in range(2):
            po = 64 * hi
            nc.tensor.transpose(pt[po : po + 64, :], cns[hi][qs], ident_sb)
        nc.vector.tensor_copy(
            ctxT_sb[:, hp, qt * 512 + qs * 128 : qt * 512 + (qs + 1) * 128], pt)

    # paired output staging: tiles (2i, 2i+1) share one [128, 2, 768] SBUF
    # buffer and leave in ONE dma (each dma_start costs ~630ns of exclusive
    # HWDGE issue time, so halving the count shortens the tail directly)
    osb_pairs = [singles.tile([128, 2, D], F16, name=f"osb{i}", tag=f"osb{i}")
                 for i in range(4)]

    def out_proj(mt, tail=False, n2s=(0, 1)):
        # out[t_tile(128), dout(768)] = sum_c ctxT16[c].T @ wo16[c] in two
        # 384-col halves. Tail tiles borrow the (dead by then) scores psum
        # pool for the second half and put that copy on ACT (idle post-exp).
        osb = osb_pairs[mt // 2]
        for n2 in n2s:
            pool = scores_psum if (tail and n2 == 1) else proj_psum
            ps = pool.tile([128, 384], FP, name="ps_o",
                           tag="proj" if pool is proj_psum else "scores")
            for c in range(MC):
                nc.tensor.matmul(
                    ps,
                    lhsT=ctxT_sb[:, c, mt * 128 : (mt + 1) * 128],
                    rhs=wo_sb[:, c, n2 * 384 : (n2 + 1) * 384],
                    start=(c == 0),
                    stop=(c == MC - 1),
                )
            csl = slice(n2 * 384, (n2 + 1) * 384)
            if tail and n2 == 0:
                # ACT is idle once the exp stream drains; splitting the two
                # halves across ACT/DVE halves the copy latency per tile
                nc.scalar.copy(osb[:, mt % 2, csl], ps)
            else:
                nc.vector.tensor_copy(osb[:, mt % 2, csl], ps)
        if 1 not in n2s:
            return
        orr = out.rearrange("(b p) d -> p b d", p=128)
        if mt >= 6:
            # last two tiles leave individually so mt6's transfer overlaps
            # mt7's compute and the final DMA is a short one
            nc.sync.dma_start(out=orr[:, mt : mt + 1, :],
                              in_=osb[:, mt % 2 : mt % 2 + 1, :])
        elif mt % 2 == 1:
            # pair complete: one DMA for rows (mt-1)*128 .. (mt+1)*128
            nc.sync.dma_start(out=orr[:, mt - 1 : mt + 1, :], in_=osb)

    # ---------------- schedule ----------------
    # Phase order staggers qt so ctx/transpose/out work spreads out:
    #   A=(0,0) B=(1,0) C=(0,1) D=(2,0) E=(1,1) F=(2,1)
    # Units within a phase run hi-major (hi0 g0..3, hi1 g0..3) so the
    # hi0 ctx units become available four exps before the phase ends.
    def phase_units(hp, qt):
        return [(hp, qt, hi, g) for hi in range(2) for g in range(3)]

    # head: critical qk m0/n0 (k copies on ACT -- idle pre-exp), phase-A
    # units interleaved with the remaining projection groups
    # head: unit (hi, g=0) needs only k/q n0; the n1 groups have two
    # exps (~3us) of slack before unit g=1 (k-tiles 3-5 span both halves)
    qk_proj(0, 0, k_on_act=True, fine=True)
    scores_unit_split(0, 0, 0, 0)
    scores_unit(0, 0, 1, 0)
    qk_proj(0, 1, dsts=("k",))
    qk_proj(0, 1, dsts=("q",))
    scores_unit(0, 0, 0, 1)
    qk_proj(1, 0, dsts=("k",))
    scores_unit(0, 0, 1, 1)
    qk_proj(1, 0, dsts=("q",))
    scores_unit(0, 0, 0, 2)
    ones_fanout()
    scores_unit(0, 0, 1, 2)

    stream = (phase_units(1, 0) + phase_units(0, 1) + phase_units(2, 0)
              + phase_units(1, 1) + phase_units(2, 1))

    # filler groups in dependency order, consumed one per scores unit
    # (each scores unit is ~0.21us PE vs a ~1.04us exp, so PE has ~0.8us
    # of filler headroom per unit)
    fillers = [
        # during B: remaining projections, v tiles
        [lambda: qk_proj(1, 1)],
        [lambda: qk_proj(2, 0)],
        [lambda: qk_proj(2, 1)],
        [lambda: v_proj([0])],
        [lambda: v_proj([1])],
        [lambda: v_proj([2])],
        # during C: rest of v, ctxA
        [lambda: v_proj([3])],
        [lambda: v_proj([4])],
        [lambda: v_proj([5])],
        [lambda: v_proj([6])],
        [lambda: v_proj([7])],
        [lambda qs=qs: ctx_q(0, 0, 0, qs) for qs in range(4)],
        # during D: ctxA hi1, ctxB, ctxC
        [lambda qs=qs: ctx_q(0, 0, 1, qs) for qs in range(4)] + [lambda: ctx_t(0, 0)],
        [lambda qs=qs: ctx_q(1, 0, 0, qs) for qs in range(4)],
        [lambda qs=qs: ctx_q(1, 0, 1, qs) for qs in range(4)] + [lambda: ctx_t(1, 0)],
        [lambda qs=qs: ctx_q(0, 1, 0, qs) for qs in range(4)],
        [lambda qs=qs: ctx_q(0, 1, 1, qs) for qs in range(4)] + [lambda: ctx_t(0, 1)],
        [],
        # during E: ctxD, out qt0 (half-tile groups: one psum+copy per
        # slot keeps the shared proj/ctx pool from head-of-line blocking
        # the scores units)
        [lambda qs=qs: ctx_q(2, 0, 0, qs) for qs in range(4)],
        [lambda qs=qs: ctx_q(2, 0, 1, qs) for qs in range(4)] + [lambda: ctx_t(2, 0)],
        [lambda: out_proj(0, n2s=(0,))],
        [lambda: out_proj(0, n2s=(1,))],
        [lambda: out_proj(1, n2s=(0,))],
        [lambda: out_proj(1, n2s=(1,))],
        # during F: ctxE, out qt0 tail, ctxF hi0
        [lambda qs=qs: ctx_q(1, 1, 0, qs) for qs in range(4)],
        [lambda qs=qs: ctx_q(1, 1, 1, qs) for qs in range(4)] + [lambda: ctx_t(1, 1)],
        [lambda: out_proj(2, n2s=(0,)), lambda: out_proj(2, n2s=(1,))],
        [lambda qs=qs: ctx_q(2, 1, 0, qs) for qs in range(4)],
        [lambda: out_proj(3, n2s=(0,))],
        [lambda: out_proj(3, n2s=(1,))],
    ]

    fi = iter(fillers)
    for u in stream[:-1]:
        scores_unit(*u)
        grp = next(fi, None)
        if grp is not None:
            for f in grp:
                f()
    # last unit is SPLIT by q-halves: its first 768-col exp covers q
    # columns 0:256 (qs 0,1), so the first two tail ctx units, transpose,
    # and out tile 4 overlap the final exp instead of following it
    scores_unit_split(2, 1, 1, 2)
    for grp in fi:
        for f in grp:
            f()

    # tail: after F's last exp only pair-2-qt1's hi1 ctx units, per-qs
    # transposes, and the qt1 out tiles remain. Emission is software-
    # pipelined so PE never waits on a DVE round-trip: while out tile qs
    # waits on its ctxT copy, PE runs ctx/transpose work for qs+1.
    ctx_q(2, 1, 1, 0)
    ctx_q(2, 1, 1, 1)
    ctx_t_qs(2, 1, 0)
    ctx_q(2, 1, 1, 2)
    ctx_t_qs(2, 1, 1)
    out_proj(4, tail=True)
    ctx_q(2, 1, 1, 3)
    ctx_t_qs(2, 1, 2)
    out_proj(5, tail=True)
    ctx_t_qs(2, 1, 3)
    out_proj(6, tail=True)
    out_proj(7, tail=True)


_PROGRAM = None


def build_program():
    global _PROGRAM
    if _PROGRAM is not None:
        return _PROGRAM
    nc = bacc.Bacc("TRN2", target_bir_lowering=False, debug=False, num_devices=NCORES)
    xh = nc.dram_tensor("xh", (D, T), F8, kind="ExternalInput").ap()
    xl = nc.dram_tensor("xl", (D, T), F8, kind="ExternalInput").ap()
    wq8 = nc.dram_tensor("wq8", (128, MC, KC, 2, 128), F8, kind="ExternalInput").ap()
    wk8 = nc.dram_tensor("wk8", (128, MC, KC, 128), F8, kind="ExternalInput").ap()
    wv8 = nc.dram_tensor("wv8", (128, KC, 2, DPC), F8, kind="ExternalInput").ap()
    wo = nc.dram_tensor("wo", (DPC, D), F16, kind="ExternalInput").ap()
    ones = nc.dram_tensor("ones", (128, HPC), F16, kind="ExternalInput").ap()
    ident = nc.dram_tensor("ident", (128, 128), F16, kind="ExternalInput").ap()
    out = nc.dram_tensor("out", (T, D), F16, kind="ExternalOutput").ap()
    from contextlib import ExitStack

    with TileContext(nc) as tc, ExitStack() as st:
        emit_mha(tc, xh, xl, wq8, wk8, wv8, wo, ones, ident, out, st)
    nc.compile()
    _PROGRAM = nc
    return nc


def _split8(a):
    hi = np.clip(a, -240.0, 240.0).astype(E4NP)
    lo = np.clip(a - hi.astype(np.float32), -240.0, 240.0).astype(E4NP)
    return hi, lo


def _pack_qk(w, both_terms=True):
    # w: [DPC, D] torch-layout slice -> packed [128, MC, KC, (2,) 128] with
    # m-chunks contiguous per partition (one >=512B-elem DMA per m-chunk)
    wt = w.T.astype(np.float32) * WSCALE          # [D, DPC]
    hi, lo = _split8(wt)
    hi = hi.reshape(KC, 128, MC, 128)
    if not both_terms:
        return np.ascontiguousarray(hi.transpose(1, 2, 0, 3))
    lo = lo.reshape(KC, 128, MC, 128)
    arr = np.stack([hi, lo], axis=0)              # [2, c, p, m, j]
    return np.ascontiguousarray(arr.transpose(2, 3, 1, 0, 4))


def _pack_v(w):
    wt = w.T.astype(np.float32) * WSCALE
    hi, lo = _split8(wt)
    arr = np.stack([hi.reshape(KC, 128, DPC), lo.reshape(KC, 128, DPC)], axis=0)
    return np.ascontiguousarray(arr.transpose(2, 1, 0, 3))  # [p, c, 2, d]


def make_in_maps(x, Wq, Wk, Wv, Wo):
    x = np.asarray(x, dtype=np.float32)
    ones = np.full((128, HPC), ONES_VAL, np.float16)
    ident = np.eye(128, dtype=np.float16)
    xs = [_split8(x[b].T) for b in range(B)]
    xs = [(np.ascontiguousarray(h), np.ascontiguousarray(l)) for h, l in xs]
    in_maps = []
    for core in range(NCORES):
        b, hh = core // 2, core % 2
        sl = slice(hh * DPC, (hh + 1) * DPC)
        in_maps.append(
            {
                "xh": xs[b][0],
                "xl": xs[b][1],
                "wq8": _pack_qk(np.asarray(Wq)[sl]),
                "wk8": _pack_qk(np.asarray(Wk)[sl], both_terms=False),
                "wv8": _pack_v(np.asarray(Wv)[sl]),
                "wo": np.ascontiguousarray(np.asarray(Wo)[:, sl].T.astype(np.float16)),
                "ones": ones,
                "ident": ident,
            }
        )
    return in_maps


def kernel(x, Wq, Wk, Wv, Wo, bo):
    nc = build_program()
    in_maps = make_in_maps(x, Wq, Wk, Wv, Wo)
    res = run_bass_kernel_spmd(nc, in_maps, core_ids=list(range(NCORES)))
    bo = np.asarray(bo, dtype=np.float32)
    out = np.empty((B, T, D), dtype=np.float32)
    for b in range(B):
        out[b] = (res.results[2 * b]["out"].astype(np.float32)
                  + res.results[2 * b + 1]["out"].astype(np.float32) + bo)
    return out
